# revision 1
# baseline (speedup 1.0000x reference)
"""2-layer GCN + dense layers + mean-pool on 8 trn2 NeuronCores (Bass/Tile).

v2 design. GCNConv out = D^-1/2 (A+I) D^-1/2 (h W) + b factorizes as
  table[v]  = (relu(x W1 + b1) Wc1)[v]          (unscaled, per node)
  agg[d]    = sum_{e: dst=d} norm_e * table[src_e],  norm_e = dinv_s*dinv_d
  h2[d]     = relu(agg[d] + b)
Self-loops are ordinary edges with norm = dinv_d^2.

Aggregation: edges bucketed by (core, src-chunk, dst-window), padded to
128-edge groups (count synced across cores for SPMD). Per group:
  - dma_gather pulls 128 table rows (fp16, 256B) into SBUF partitions
  - one DVE dual-op tensor_scalar builds a norm-valued one-hot:
      oh[e, slot] = (iota==dloc_e) * norm_e
  - one matmul out += rows.T @ oh -> PSUM window [128 feat, 128 slot]
    (feature-major!), accumulated over all groups of the window
  - one activation evacuates: h2T[:, w] = relu(win + bias), feature-major,
    which is exactly the layout the next dense stage consumes.
No transposes, no post-scale pass anywhere.

Table layout: row f(v) = (v%128)*784 + v//128 so the dense stage writes
8 node-tiles per DMA with 2048B contiguous per partition. Gather indices
are int16 relative to 32767-row chunk slices of the single table tensor.
Reserved zero rows (u=783 stripe) serve as pad-gather targets.

Launch 1: dense D1 (all nodes, replicated per core) -> conv1 table ->
aggregate own dst shard -> h2T -> dense D2 -> g2s rows (conv2 table shard,
rows f2 = slot*98 + w). Host concatenates shards into the conv2 table.
Launch 2: aggregate conv2 -> h4T -> D3 + graph-pool partials. Host sums.

Dst nodes are re-binned across (core, window) to balance per-bucket edge
counts (reduces group padding); all index bookkeeping is host-side.
"""

import os
import sys

sys.path.insert(0, "/opt/trn_rl_repo")

import contextlib

import numpy as np

import concourse.bass as bass
import concourse.tile as tile
from concourse import bacc, mybir
from concourse.bass_utils import run_bass_kernel_spmd

F32 = mybir.dt.float32
F16 = mybir.dt.float16
I16 = mybir.dt.int16
AF = mybir.ActivationFunctionType
ALU = mybir.AluOpType

N = 100000
F = 128
NOUT = 64
NG = 64
NCORES = 8
WIN = 128
WPC = 98                    # windows per core
SHPAD = WPC * WIN           # 12544 padded shard nodes per core
NBINS = NCORES * WPC        # 784 dst bins
NU = 784                    # u values: v = p*? ... f(v) = (v%128)*NU + v//128
TAB = 128 * NU              # 100352 table rows
CRE = 25088                 # rows per gather chunk (= 32*784 = 32*98*8,
                            # int16 idx range; L1 chunk = p//32, L2 = slot//32)
NCHUNK = 4
GCALL = 32                  # gather groups per dma_gather call

LAST_EXEC_NS = None
LAST_INFO = {}


def _f_of_v(v):
    v = np.asarray(v, np.int64)
    return (v % 128) * NU + v // 128


# ----------------------------------------------------------------------------
# host-side graph prep
# ----------------------------------------------------------------------------
def _prep(src, dst, batch):
    src = np.asarray(src, np.int64)
    dst = np.asarray(dst, np.int64)
    batch = np.asarray(batch, np.int64)

    deg = np.bincount(dst, minlength=N).astype(np.float64) + 1.0
    dinv = (1.0 / np.sqrt(deg)).astype(np.float64)

    loops = np.arange(N, dtype=np.int64)
    s_all = np.concatenate([src, loops])
    d_all = np.concatenate([dst, loops])
    norm_all = (dinv[s_all] * dinv[d_all]).astype(np.float32)
    E = len(s_all)

    frow = _f_of_v(s_all)               # table row of src
    chunk = frow // CRE                 # gather chunk
    iloc = (frow % CRE).astype(np.int16)

    # --- balanced dst binning -------------------------------------------
    # Window classes G5/G6 with per-chunk caps just under G*128 so that
    # ceil(max_core cnt/128) == G; greedy assignment (random order) with a
    # deviation-corrective score keeps every (core,chunk,window) bucket
    # under its cap. Caps are set conservatively so L2's (unsteered)
    # bucket counts also stay under the same multiples.
    d4 = np.zeros((N, NCHUNK), np.int64)
    np.add.at(d4, (d_all, chunk), 1)
    degv = d4.sum(1)
    MARGIN = 25
    capG = {4: 4 * 128 - MARGIN, 5: 5 * 128 - MARGIN, 6: 6 * 128 - MARGIN}
    share = d4.sum(0).max() / degv.sum()   # ~0.25 per (equal) chunk
    Ecore = degv.sum() / NCORES * 1.004
    Ty = capG[4] / share
    Tz = capG[5] / share
    z = int(np.ceil(max(0.0, (Ecore - WPC * Ty) / (Tz - Ty))))
    z = min(z, WPC)
    wclass = np.array([5] * z + [4] * (WPC - z))
    caps = np.zeros((NBINS, NCHUNK), np.float64)
    for b in range(NBINS):
        caps[b, :] = capG[wclass[b % WPC]]
    rem = caps.copy()
    mu = caps / 128.0
    slots = np.full(NBINS, 128, np.float64)
    rng = np.random.default_rng(0)
    order = rng.permutation(N)
    bin_of = np.full(N, -1, np.int64)
    for v in order:
        need = d4[v]
        ok = (rem >= need).all(1) & (slots > 0)
        if not ok.any():
            ok = slots > 0
        dev = rem - need - (slots[:, None] - 1) * mu
        sc = np.where(ok, (dev * dev).sum(1), np.inf)
        b = int(np.argmin(sc))
        bin_of[v] = b
        rem[b] -= need
        slots[b] -= 1
    # slots within bin: match each node's slot-quarter (= its L2 chunk)
    # to its p-quarter (= its L1 chunk) so L2 bucket counts track the
    # steered L1 counts; spill overfull quarters to least-full ones
    slot_of = np.empty(N, np.int64)
    occ = np.bincount(bin_of, minlength=NBINS)
    sidx = np.argsort(bin_of, kind="stable")
    starts = np.concatenate([[0], np.cumsum(occ)])
    pq = (np.arange(N) % 128) // 32        # p-quarter of node id
    for b in range(NBINS):
        vs = sidx[starts[b]:starts[b + 1]]
        qfill = [0, 0, 0, 0]
        spill = []
        for v in vs:
            q = int(pq[v])
            if qfill[q] < 32:
                slot_of[v] = q * 32 + qfill[q]
                qfill[q] += 1
            else:
                spill.append(v)
        for v in spill:
            q = int(np.argmin(qfill))
            slot_of[v] = q * 32 + qfill[q]
            qfill[q] += 1

    core_of = bin_of // WPC
    w_of = bin_of % WPC

    e_core = core_of[d_all]
    e_w = w_of[d_all]
    e_slot = slot_of[d_all].astype(np.float32)

    # --- group schedule: G[ch, w] = ceil(max_core count / 128) -------------
    key = ((e_core * NCHUNK + chunk) * WPC + e_w)
    nk = NCORES * NCHUNK * WPC
    cnt = np.bincount(key, minlength=nk).reshape(NCORES, NCHUNK, WPC)
    G = np.ceil(cnt.max(axis=0) / 128.0).astype(np.int64)   # [NCHUNK, WPC]
    Gc = G.sum(axis=1)                  # groups per chunk
    GT = int(G.sum())
    EPAD = GT * 128

    # emission order is (w, ch, j); chunk-local gather order is (w, j).
    # global gidx of (w, ch, j):
    gbase = np.zeros((WPC, NCHUNK), np.int64)     # start gidx of (w, ch)
    run = 0
    for w in range(WPC):
        for ch in range(NCHUNK):
            gbase[w, ch] = run
            run += G[ch, w]
    assert run == GT
    # chunk-local group offset of (w, ch):
    cbase = np.zeros((WPC, NCHUNK), np.int64)
    crun = np.zeros(NCHUNK, np.int64)
    for w in range(WPC):
        for ch in range(NCHUNK):
            cbase[w, ch] = crun[ch]
            crun[ch] += G[ch, w]
    assert (crun == Gc).all()

    # pad gather target: reserved zero rows are f = p*NU + 783
    pad_iloc = np.zeros(NCHUNK, np.int64)
    for ch in range(NCHUNK):
        p0 = -(-(ch * CRE - 783) // NU)           # smallest p with row>=base
        p0 = max(p0, 0)
        r = p0 * NU + 783
        assert ch * CRE <= r < min((ch + 1) * CRE, TAB), (ch, r)
        pad_iloc[ch] = r - ch * CRE

    # --- fill streams ------------------------------------------------------
    eorder = np.lexsort((chunk, e_w, e_core))
    key_s = ((e_core * WPC + e_w) * NCHUNK + chunk)[eorder]
    iloc_s = iloc[eorder]
    slot_s = e_slot[eorder]
    norm_s = norm_all[eorder]
    bounds = np.searchsorted(key_s, np.arange(NCORES * WPC * NCHUNK + 1))

    idx_streams = np.zeros((NCORES, NCHUNK, max(int(c) for c in Gc) * 128),
                           np.int16)
    for ch in range(NCHUNK):
        idx_streams[:, ch, :] = pad_iloc[ch]
    dloc2d = np.full((NCORES, 128, GT), -1.0, np.float32)
    norm2d = np.zeros((NCORES, 128, GT), np.float32)
    for c in range(NCORES):
        for w in range(WPC):
            for ch in range(NCHUNK):
                k = (c * WPC + w) * NCHUNK + ch
                b0, b1 = bounds[k], bounds[k + 1]
                n = b1 - b0
                g = int(G[ch, w])
                assert n <= g * 128
                co = int(cbase[w, ch]) * 128
                idx_streams[c, ch, co:co + n] = iloc_s[b0:b1]
                gg = int(gbase[w, ch])
                sl = np.full(g * 128, -1.0, np.float32)
                nv = np.zeros(g * 128, np.float32)
                sl[:n] = slot_s[b0:b1]
                nv[:n] = norm_s[b0:b1]
                dloc2d[c, :, gg:gg + g] = sl.reshape(g, 128).T
                norm2d[c, :, gg:gg + g] = nv.reshape(g, 128).T

    # idx SBUF layout per chunk: [128, len/16] replicated over 8 part-groups
    idx2d = np.zeros((NCORES, NCHUNK, 128, idx_streams.shape[2] // 16),
                     np.int16)
    for c in range(NCORES):
        for ch in range(NCHUNK):
            a = idx_streams[c, ch].reshape(-1, 16).T
            idx2d[c, ch] = np.tile(a, (8, 1))

    # --- L2 bookkeeping ----------------------------------------------------
    # conv2 table row of node v: (slot*WPC + w)*NCORES + core, interleaving
    # cores so L2 chunk buckets stay balanced across cores
    row2 = (slot_of * WPC + w_of) * NCORES + core_of
    frow2 = row2[s_all]

    # zero/pad rows for table2: unoccupied (c,w,slot) from the used mask
    used = np.zeros((NBINS, WIN), bool)
    used[bin_of, slot_of] = True
    ub, us = np.nonzero(~used)
    unocc = np.sort(((us * WPC + (ub % WPC)) * NCORES + ub // WPC))

    counts = np.maximum(np.bincount(batch, minlength=NG), 1).astype(np.float64)
    g2d = np.zeros((NCORES, 128, WPC * NG), np.float16)
    nodes = np.arange(N)
    for c in range(NCORES):
        m = core_of == c
        vv = nodes[m]
        g2d[c, slot_of[vv], w_of[vv] * NG + batch[vv]] = (
            1.0 / counts[batch[vv]]).astype(np.float16)

    return dict(
        G=G, Gc=Gc, GT=GT, gbase=gbase, cbase=cbase,
        idx2d=idx2d, dloc2d=dloc2d, norm2d=norm2d,
        core_of=core_of, w_of=w_of, slot_of=slot_of, occ=occ,
        row2=row2, unocc=unocc, counts=counts, g2d=g2d,
        s_all=s_all, d_all=d_all, norm_all=norm_all,
        e_core=e_core, e_w=e_w, e_slot=e_slot, chunk=chunk,
        eorder=eorder, bounds=bounds, frow2=frow2,
    )


def _prep_l2_idx(prep):
    """L2 gather streams: same (w, j) group layout but bucketed by chunk2 =
    frow2 // CRE, which differs per edge from L1's chunk. Rebuild schedule."""
    frow2 = prep["frow2"]
    chunk2 = frow2 // CRE
    iloc2 = (frow2 % CRE).astype(np.int16)
    e_core, e_w = prep["e_core"], prep["e_w"]
    e_slot, norm_all = prep["e_slot"], prep["norm_all"]

    key = ((e_core * NCHUNK + chunk2) * WPC + e_w)
    nk = NCORES * NCHUNK * WPC
    cnt = np.bincount(key, minlength=nk).reshape(NCORES, NCHUNK, WPC)
    G = np.ceil(cnt.max(axis=0) / 128.0).astype(np.int64)
    Gc = G.sum(axis=1)
    GT = int(G.sum())

    gbase = np.zeros((WPC, NCHUNK), np.int64)
    run = 0
    for w in range(WPC):
        for ch in range(NCHUNK):
            gbase[w, ch] = run
            run += G[ch, w]
    cbase = np.zeros((WPC, NCHUNK), np.int64)
    crun = np.zeros(NCHUNK, np.int64)
    for w in range(WPC):
        for ch in range(NCHUNK):
            cbase[w, ch] = crun[ch]
            crun[ch] += G[ch, w]

    pad2_row = np.zeros(NCHUNK, np.int64)
    unocc = prep["unocc"]
    for ch in range(NCHUNK):
        lo, hi = ch * CRE, min((ch + 1) * CRE, NCORES * SHPAD)
        cand = unocc[(unocc >= lo) & (unocc < hi)]
        assert len(cand) > 0
        pad2_row[ch] = cand[0]

    eorder = np.lexsort((chunk2, e_w, e_core))
    key_s = ((e_core * WPC + e_w) * NCHUNK + chunk2)[eorder]
    iloc_s = iloc2[eorder]
    slot_s = e_slot[eorder]
    norm_s = norm_all[eorder]
    bounds = np.searchsorted(key_s, np.arange(NCORES * WPC * NCHUNK + 1))

    mg = max(int(c) for c in Gc) * 128
    idx_streams = np.zeros((NCORES, NCHUNK, mg), np.int16)
    for ch in range(NCHUNK):
        idx_streams[:, ch, :] = pad2_row[ch] - ch * CRE
    dloc2d = np.full((NCORES, 128, GT), -1.0, np.float32)
    norm2d = np.zeros((NCORES, 128, GT), np.float32)
    for c in range(NCORES):
        for w in range(WPC):
            for ch in range(NCHUNK):
                k = (c * WPC + w) * NCHUNK + ch
                b0, b1 = bounds[k], bounds[k + 1]
                n = b1 - b0
                g = int(G[ch, w])
                assert n <= g * 128
                co = int(cbase[w, ch]) * 128
                idx_streams[c, ch, co:co + n] = iloc_s[b0:b1]
                gg = int(gbase[w, ch])
                sl = np.full(g * 128, -1.0, np.float32)
                nv = np.zeros(g * 128, np.float32)
                sl[:n] = slot_s[b0:b1]
                nv[:n] = norm_s[b0:b1]
                dloc2d[c, :, gg:gg + g] = sl.reshape(g, 128).T
                norm2d[c, :, gg:gg + g] = nv.reshape(g, 128).T

    idx2d = np.zeros((NCORES, NCHUNK, 128, mg // 16), np.int16)
    for c in range(NCORES):
        for ch in range(NCHUNK):
            a = idx_streams[c, ch].reshape(-1, 16).T
            idx2d[c, ch] = np.tile(a, (8, 1))

    return dict(G=G, Gc=Gc, GT=GT, gbase=gbase, cbase=cbase,
                idx2d=idx2d, dloc2d=dloc2d, norm2d=norm2d)


# ----------------------------------------------------------------------------
# device program pieces
# ----------------------------------------------------------------------------
def _preload_idx(nc, tc, ctx, idx_aps, sched):
    """Load gather index streams into SBUF (one tile per chunk)."""
    Gc = sched["Gc"]
    idxc = ctx.enter_context(tc.tile_pool(name="idxc", bufs=1))
    idx_sb = {}
    for ch in range(NCHUNK):
        if Gc[ch] == 0:
            continue
        it = idxc.tile([128, int(Gc[ch]) * 8], I16, tag=f"idx{ch}")
        nc.sync.dma_start(it[:], idx_aps[ch])
        idx_sb[ch] = it
    return idx_sb


def _emit_agg(nc, tc, ctx, tab_ap, idx_aps, dloc_sb, norm_sb, iota_sb,
              bias_sb, hT, sched, winps, tab_nrows, on_window=None,
              idx_sb=None):
    """Aggregate edges; hT[:, w*128:(w+1)*128] = relu(agg_w + bias),
    feature-major. sched = dict(G, Gc, gbase, cbase). tab_ap: [rows, F] f16;
    idx_aps[ch]: DRAM idx AP [128, Gc[ch]*8]. on_window(w) is called after
    window w's activation is emitted (for fused downstream consumers)."""
    G, Gc = sched["G"], sched["Gc"]

    if idx_sb is None:
        idx_sb = _preload_idx(nc, tc, ctx, idx_aps, sched)
    gath = {}
    for ch in range(NCHUNK):
        if Gc[ch] == 0:
            continue
        gath[ch] = ctx.enter_context(
            tc.tile_pool(name=f"gath{ch}", bufs=3))

    # issue gather calls lazily per chunk as windows consume groups;
    # taper the final calls so the post-DMA compute drain is short
    tiles = {ch: [] for ch in range(NCHUNK)}     # (tile, goff, ng)
    issued = {ch: 0 for ch in range(NCHUNK)}

    def ensure(ch, upto):
        while issued[ch] <= upto:
            g0 = issued[ch]
            rem = Gc[ch] - g0
            ng = int(min(GCALL if rem > 2 * GCALL else GCALL // 4, rem))
            gt = gath[ch].tile([128, GCALL * F], F16, tag="gt")
            base = ch * CRE
            hi = min(base + CRE, tab_nrows)
            nc.gpsimd.dma_gather(
                gt[:, :ng * F].rearrange("p (g e) -> p g e", e=F),
                tab_ap[base:hi, :],
                idx_sb[ch][:, g0 * 8:(g0 + ng) * 8], ng * 128, ng * 128, F,
                single_packet=False,
            )
            tiles[ch].append((gt, g0, ng))
            issued[ch] += ng

    ohp = ctx.enter_context(tc.tile_pool(name="ohp", bufs=6))
    gbase, cbase = sched["gbase"], sched["cbase"]
    for w in range(WPC):
        wt = winps.tile([128, 128], F32, tag="win")
        total = int(sum(G[ch, w] for ch in range(NCHUNK)))
        done = 0
        for ch in range(NCHUNK):
            g = int(G[ch, w])
            for j in range(g):
                cg = int(cbase[w, ch]) + j      # chunk-local group
                ensure(ch, cg)
                gt, g0, ng = next(
                    t for t in tiles[ch] if t[1] <= cg < t[1] + t[2])
                k = cg - g0
                gg = int(gbase[w, ch]) + j      # global gidx
                oh = ohp.tile([128, 128], F16, tag="oh")
                nc.vector.tensor_scalar(
                    oh[:], iota_sb[:], dloc_sb[:, gg:gg + 1],
                    norm_sb[:, gg:gg + 1], ALU.is_equal, ALU.mult,
                )
                nc.tensor.matmul(
                    wt[:], gt[:, k * F:(k + 1) * F], oh[:],
                    start=(done == 0), stop=(done == total - 1),
                )
                done += 1
        if total == 0:
            nc.vector.memset(wt[:], 0.0)
        nc.scalar.activation(hT[:, w * F:(w + 1) * F], wt[:], AF.Relu,
                             bias=bias_sb[:, 0:1])
        if on_window is not None:
            on_window(w)


def _emit_dense(nc, tc, ctx, nti, src_get, wa_sb, wb_sb, ba_sb, out_wr,
                mm1ps, tabps, hpool, stpool, last_partial=0):
    """Generic 2-matmul dense chain, feature-major in, node-major out.

    nti: number of 512-node tiles. src_get(u512) -> SBUF [128, 512] f16
    feature-major input slice. out_wr(u0, nu, stage_tile) writes rows
    [128, nu*256B] (u-major batches of 8 tiles).
    last_partial: if >0, zero the last `last_partial` u-slices of the final
    batch (reserved zero rows).
    """
    for u512 in range(nti):
        xt = src_get(u512)
        p1 = mm1ps.tile([128, 512], F32, tag="p1")
        nc.tensor.matmul(p1[:], wa_sb[:], xt, start=True, stop=True)
        h1 = hpool.tile([128, 512], F16, tag="h1")
        nc.scalar.activation(h1[:], p1[:], AF.Relu, bias=ba_sb[:, 0:1])
        q8 = u512 % 2
        if q8 == 0:
            tp = tabps.tile([128, 1024], F32, tag="tp")
            _emit_dense.tp = tp
        tp = _emit_dense.tp
        for q in range(4):
            nc.tensor.matmul(
                tp[:, (q8 * 4 + q) * 128:(q8 * 4 + q + 1) * 128],
                h1[:, q * 128:(q + 1) * 128], wb_sb[:],
                start=True, stop=True,
            )
        if q8 == 1 or u512 == nti - 1:
            nu = (q8 + 1) * 4
            st = stpool.tile([128, 1024], F16, tag="st")
            nc.vector.tensor_copy(st[:, :nu * 128], tp[:, :nu * 128])
            if u512 == nti - 1 and last_partial:
                nc.vector.memset(
                    st[:, (nu - last_partial) * 128:nu * 128], 0.0)
            out_wr((u512 // 2) * 8, nu, st)


# ----------------------------------------------------------------------------
# builders
# ----------------------------------------------------------------------------
def _build_launch1(prep):
    nc = bacc.Bacc("TRN2", target_bir_lowering=False, debug=False,
                   num_devices=NCORES)
    GT = prep["GT"]
    Gc = prep["Gc"]
    NXC = TAB // 512                       # 196 x-tiles of 512

    xT = nc.dram_tensor("xT", [128, TAB], F16, kind="ExternalInput")
    w1 = nc.dram_tensor("w1", [128, 128], F16, kind="ExternalInput")
    wc1 = nc.dram_tensor("wc1", [128, 128], F16, kind="ExternalInput")
    wfc2 = nc.dram_tensor("wfc2", [128, 128], F16, kind="ExternalInput")
    wc2 = nc.dram_tensor("wc2", [128, 128], F16, kind="ExternalInput")
    b1 = nc.dram_tensor("b1", [128, 1], F32, kind="ExternalInput")
    bc1 = nc.dram_tensor("bc1", [128, 1], F32, kind="ExternalInput")
    bfc2 = nc.dram_tensor("bfc2", [128, 1], F32, kind="ExternalInput")
    idxs = [nc.dram_tensor("idx%d" % ch, [128, int(Gc[ch]) * 8], I16,
                           kind="ExternalInput") for ch in range(NCHUNK)]
    dloc = nc.dram_tensor("dloc", [128, GT], F32, kind="ExternalInput")
    norm = nc.dram_tensor("norm", [128, GT], F32, kind="ExternalInput")
    iota = nc.dram_tensor("iota", [128, 128], F16, kind="ExternalInput")
    g2s = nc.dram_tensor("g2s", [SHPAD, F], F16, kind="ExternalOutput")

    with tile.TileContext(nc) as tc, contextlib.ExitStack() as ctx:
        dram = ctx.enter_context(tc.tile_pool(name="dram", bufs=1,
                                              space="DRAM"))
        tabt = dram.tile([TAB, F], F16, tag="tab")
        tab_pm = tabt[:].rearrange("(p u) f -> p (u f)", p=128)

        const = ctx.enter_context(tc.tile_pool(name="const", bufs=1))
        big = ctx.enter_context(tc.tile_pool(name="big", bufs=1))
        _n = [0]

        def ld(ap, shape, dtype):
            _n[0] += 1
            t = const.tile(shape, dtype, tag="c%d" % _n[0])
            nc.sync.dma_start(t[:], ap)
            return t

        w1_sb = ld(w1.ap(), [128, 128], F16)
        wc1_sb = ld(wc1.ap(), [128, 128], F16)
        wfc2_sb = ld(wfc2.ap(), [128, 128], F16)
        wc2_sb = ld(wc2.ap(), [128, 128], F16)
        b1_sb = ld(b1.ap(), [128, 1], F32)
        bc1_sb = ld(bc1.ap(), [128, 1], F32)
        bfc2_sb = ld(bfc2.ap(), [128, 1], F32)
        dloc_sb = ld(dloc.ap(), [128, GT], F32)
        norm_sb = ld(norm.ap(), [128, GT], F32)
        iota_sb = ld(iota.ap(), [128, 128], F16)
        h2T = big.tile([128, SHPAD], F16)

        mm1ps = ctx.enter_context(tc.tile_pool(name="mm1ps", bufs=2,
                                               space="PSUM"))
        tabps = ctx.enter_context(tc.tile_pool(name="tabps", bufs=2,
                                               space="PSUM"))
        hpool = ctx.enter_context(tc.tile_pool(name="hpool", bufs=3))
        stpool = ctx.enter_context(tc.tile_pool(name="stpool", bufs=3))
        xin = ctx.enter_context(tc.tile_pool(name="xin", bufs=3))

        # ---- D1: conv1 table --------------------------------------------
        xt_cache = {}

        def src1(u512):
            blk = u512 // 4                # 2048-col loads
            if blk not in xt_cache:
                t = xin.tile([128, 2048], F16, tag="xt")
                nc.scalar.dma_start(t[:], xT.ap()[:, blk * 2048:(blk + 1) * 2048])
                xt_cache.clear()
                xt_cache[blk] = t
            return xt_cache[blk][:, (u512 % 4) * 512:(u512 % 4 + 1) * 512]

        def wr1(u0, nu, st):
            nc.sync.dma_start(tab_pm[:, u0 * 128:(u0 + nu) * 128],
                              st[:, :nu * 128])

        # zero reserved rows (u=783) first: they are covered by the last
        # write batch (u 776..783) whose final u-slice is zeroed via
        # last_partial=1 (v >= 100224 is padding anyway).
        _emit_dense(nc, tc, ctx, NXC, src1, w1_sb, wc1_sb, b1_sb, wr1,
                    mm1ps, tabps, hpool, stpool, last_partial=1)

        # ---- aggregate conv1, D2 fused per 4 windows --------------------
        winps = ctx.enter_context(tc.tile_pool(name="winps", bufs=2,
                                               space="PSUM"))
        g2s_pm = g2s.ap().rearrange("(p u) f -> p (u f)", p=128)  # u2=WPC=98
        d2state = {}

        def d2_tile(w):
            # after window w: if a full 512-col slice [u512*512 ..) is ready,
            # run the D2 dense chain on it
            if w % 4 != 3 and w != WPC - 1:
                return
            u512 = w // 4
            c0 = u512 * 512
            cw = min(512, SHPAD - c0)
            p1 = mm1ps.tile([128, 512], F32, tag="p1")
            nc.tensor.matmul(p1[:, :cw], wfc2_sb[:], h2T[:, c0:c0 + cw],
                             start=True, stop=True)
            h1 = hpool.tile([128, 512], F16, tag="h1")
            nc.scalar.activation(h1[:, :cw], p1[:, :cw], AF.Relu,
                                 bias=bfc2_sb[:, 0:1])
            q8 = u512 % 2
            if q8 == 0:
                tp_new = tabps.tile([128, 1024], F32, tag="tp")
                d2state["tp"] = tp_new
            tp2 = d2state["tp"]
            for q in range(cw // 128):
                nc.tensor.matmul(
                    tp2[:, (q8 * 4 + q) * 128:(q8 * 4 + q + 1) * 128],
                    h1[:, q * 128:(q + 1) * 128], wc2_sb[:],
                    start=True, stop=True,
                )
            if q8 == 1 or u512 == 24:
                u0 = (u512 // 2) * 8
                nu = min(q8 * 4 + cw // 128, WPC - u0)
                st = stpool.tile([128, 1024], F16, tag="st")
                nc.vector.tensor_copy(st[:, :nu * 128], tp2[:, :nu * 128])
                nc.sync.dma_start(g2s_pm[:, u0 * 128:(u0 + nu) * 128],
                                  st[:, :nu * 128])

        _emit_agg(nc, tc, ctx, tabt[:], [a.ap() for a in idxs], dloc_sb,
                  norm_sb, iota_sb, bc1_sb, h2T, prep, winps, TAB,
                  on_window=d2_tile)

    nc.compile()
    return nc


def _build_launch2(prep, prep2):
    nc = bacc.Bacc("TRN2", target_bir_lowering=False, debug=False,
                   num_devices=NCORES)
    GT = prep2["GT"]
    Gc = prep2["Gc"]
    T2 = NCORES * SHPAD                    # 100352 rows

    tab2 = nc.dram_tensor("tab2", [T2, F], F16, kind="ExternalInput")
    wfc = nc.dram_tensor("wfc", [128, NOUT], F16, kind="ExternalInput")
    bc2 = nc.dram_tensor("bc2", [128, 1], F32, kind="ExternalInput")
    idxs = [nc.dram_tensor("idx%d" % ch, [128, int(Gc[ch]) * 8], I16,
                           kind="ExternalInput") for ch in range(NCHUNK)]
    dloc = nc.dram_tensor("dloc", [128, GT], F32, kind="ExternalInput")
    norm = nc.dram_tensor("norm", [128, GT], F32, kind="ExternalInput")
    g2d = nc.dram_tensor("g2d", [128, WPC * NG], F16, kind="ExternalInput")
    iota = nc.dram_tensor("iota", [128, 128], F16, kind="ExternalInput")
    pool = nc.dram_tensor("pool", [NG, NOUT], F32, kind="ExternalOutput")

    with tile.TileContext(nc) as tc, contextlib.ExitStack() as ctx:
        const = ctx.enter_context(tc.tile_pool(name="const", bufs=1))
        big = ctx.enter_context(tc.tile_pool(name="big", bufs=1))
        _n = [0]

        def ld(ap, shape, dtype):
            _n[0] += 1
            t = const.tile(shape, dtype, tag="c%d" % _n[0])
            nc.sync.dma_start(t[:], ap)
            return t

        # idx streams first: the first gather only needs these (+table)
        idx_sb = _preload_idx(nc, tc, ctx, [a.ap() for a in idxs], prep2)
        wfc_sb = ld(wfc.ap(), [128, NOUT], F16)
        bc2_sb = ld(bc2.ap(), [128, 1], F32)
        iota_sb = ld(iota.ap(), [128, 128], F16)
        dloc_sb = ld(dloc.ap(), [128, GT], F32)
        norm_sb = ld(norm.ap(), [128, GT], F32)
        g2d_sb = ld(g2d.ap(), [128, WPC * NG], F16)
        h4T = big.tile([128, SHPAD], F16)

        winps = ctx.enter_context(tc.tile_pool(name="winps", bufs=2,
                                               space="PSUM"))
        psd = ctx.enter_context(tc.tile_pool(name="psd", bufs=3, space="PSUM"))
        osb = ctx.enter_context(tc.tile_pool(name="osb", bufs=4))
        psp = ctx.enter_context(tc.tile_pool(name="psp", bufs=1, space="PSUM"))
        poolps = psp.tile([NG, NOUT], F32)

        def d3_win(w):
            pd = psd.tile([128, NOUT], F32, tag="pd")
            nc.tensor.matmul(pd[:], h4T[:, w * F:(w + 1) * F], wfc_sb[:],
                             start=True, stop=True)
            ot = osb.tile([128, NOUT], F16, tag="ot")
            nc.scalar.activation(ot[:], pd[:], AF.Copy)
            nc.tensor.matmul(poolps[:], g2d_sb[:, w * NG:(w + 1) * NG],
                             ot[:], start=(w == 0), stop=(w == WPC - 1),
                             skip_group_check=True)

        _emit_agg(nc, tc, ctx, tab2.ap(), [a.ap() for a in idxs], dloc_sb,
                  norm_sb, iota_sb, bc2_sb, h4T, prep2, winps, T2,
                  on_window=d3_win, idx_sb=idx_sb)
        pres = osb.tile([NG, NOUT], F32, tag="pres")
        nc.vector.tensor_copy(pres[:], poolps[:])
        nc.sync.dma_start(pool.ap(), pres[:])

    nc.compile()
    return nc


def _np16(x):
    return np.ascontiguousarray(x, np.float16)


def kernel(x, src, dst, batch, W_fc1, b_fc1, W_c1, b_c1, W_fc2, b_fc2, W_c2,
           b_c2, W_fc, b_fc):
    global LAST_EXEC_NS, LAST_INFO
    x = np.asarray(x, np.float32)
    prep = _prep(src, dst, batch)
    prep2 = _prep_l2_idx(prep)
    trace = os.environ.get("KERNEL_TRACE", "0") == "1"

    # xT in table order: col f(v)... no: dense iterates v-order; xT col = v.
    xT = np.zeros((128, TAB), np.float16)
    xT[:, :N] = x.T
    col = lambda b: np.ascontiguousarray(
        np.asarray(b, np.float32).reshape(-1, 1))
    iota = np.tile(np.arange(128, dtype=np.float16), (128, 1))

    nc1 = _build_launch1(prep)
    in_maps1 = []
    for c in range(NCORES):
        im = {
            "xT": xT, "w1": _np16(W_fc1), "wc1": _np16(W_c1),
            "wfc2": _np16(W_fc2), "wc2": _np16(W_c2),
            "b1": col(b_fc1), "bc1": col(b_c1), "bfc2": col(b_fc2),
            "dloc": prep["dloc2d"][c], "norm": prep["norm2d"][c],
            "iota": iota,
        }
        for ch in range(NCHUNK):
            gc = int(prep["Gc"][ch]) * 8
            im["idx%d" % ch] = np.ascontiguousarray(
                prep["idx2d"][c, ch][:, :gc])
        in_maps1.append(im)
    r1 = run_bass_kernel_spmd(nc1, in_maps1, core_ids=list(range(NCORES)),
                              trace=trace)
    t1_ns = None
    if os.environ.get("KERNEL_TIME", "0") == "1":
        from concourse.timeline_sim import TimelineSim
        tl = TimelineSim(nc1, trace=False)
        tl.simulate()
        t1_ns = int(tl.time)

    # host assemble conv2 table: row (slot*WPC + w)*NCORES + core
    tab2 = np.zeros((NCORES * SHPAD, F), np.float16)
    t2v = tab2.reshape(SHPAD, NCORES, F)
    for c in range(NCORES):
        t2v[:, c, :] = r1.results[c]["g2s"]
    tab2[prep["unocc"]] = 0.0

    nc2 = _build_launch2(prep, prep2)
    in_maps2 = []
    for c in range(NCORES):
        im = {
            "tab2": tab2, "wfc": _np16(W_fc), "bc2": col(b_c2),
            "dloc": prep2["dloc2d"][c], "norm": prep2["norm2d"][c],
            "g2d": prep["g2d"][c], "iota": iota,
        }
        for ch in range(NCHUNK):
            gc = int(prep2["Gc"][ch]) * 8
            im["idx%d" % ch] = np.ascontiguousarray(
                prep2["idx2d"][c, ch][:, :gc])
        in_maps2.append(im)
    r2 = run_bass_kernel_spmd(nc2, in_maps2, core_ids=list(range(NCORES)),
                              trace=trace)
    t2_ns = None
    if os.environ.get("KERNEL_TIME", "0") == "1":
        from concourse.timeline_sim import TimelineSim
        tl = TimelineSim(nc2, trace=False)
        tl.simulate()
        t2_ns = int(tl.time)

    out = np.zeros((NG, NOUT), np.float64)
    for c in range(NCORES):
        out += r2.results[c]["pool"].astype(np.float64)
    out = out + np.asarray(b_fc, np.float64)[None, :]

    t1 = r1.exec_time_ns or t1_ns
    t2 = r2.exec_time_ns or t2_ns
    LAST_EXEC_NS = (t1 or 0) + (t2 or 0)
    LAST_INFO = {"t1": t1, "t2": t2, "GT1": prep["GT"], "GT2": prep2["GT"]}
    return out.astype(np.float32)



# revision 4
# speedup vs baseline: 1.2455x; 1.2455x over previous
"""2-layer GCN + dense layers + mean-pool on 8 trn2 NeuronCores (Bass/Tile).

v3 design (3 launches, sharded dense, self-loops via local diag matmuls).

GCNConv out = D^-1/2 (A+I) D^-1/2 (h W) + b factorizes as
  table[v]  = (h W)[v]                      (unscaled, per node)
  agg[d]    = sum_{e: dst=d} norm_e * table[src_e] + dinv_d^2 * table[d]
  h2[d]     = relu(agg[d] + b)
Self-loop terms never enter the gather stream: each core keeps its own dst
shard's table rows (slot-major, contiguous) in SBUF and adds them with one
matmul per window against a DVE-built diagonal (values dinv_d^2).

Node placement: greedy binning assigns each node to a (core, window) bin
(128 slots each) with per-chunk caps, where chunk class = node_id % 4 and
slots are class quarters (slot = class*32 + sloc).  Table row of node v:
  row(v) = class*25088 + core*3136 + sloc*98 + w   (= core shard row s*98+w)
so each core's dense output shard [12544, 128] is contiguous, gather chunks
(int16 idx) are fixed row ranges, and host-side assembly is pure reshape.

Launch 1: per-core dense D1 (x shard -> relu(xW1+b1) Wc1) -> shard rows.
Host: assemble table1, build own-shard views.  Launch 2: aggregate conv1
(dma_gather per 128-edge group + one-hot matmul, feature-major windows),
fused D2 -> table2 shard rows.  Host: assemble table2.  Launch 3: aggregate
conv2 + D3 + graph-pool partials; host sums partials + b_fc.

Both conv layers share one gather schedule (same graph, same row map).
"""

import os
import sys

sys.path.insert(0, "/opt/trn_rl_repo")

import contextlib

import numpy as np

import concourse.bass as bass
import concourse.tile as tile
from concourse import bacc, mybir
from concourse.bass_utils import run_bass_kernel_spmd

F32 = mybir.dt.float32
F16 = mybir.dt.float16
I16 = mybir.dt.int16
AF = mybir.ActivationFunctionType
ALU = mybir.AluOpType

N = 100000
F = 128
NOUT = 64
NG = 64
NCORES = 8
WPC = 98                    # windows per core
WIN = 128
SH = WPC * WIN              # 12544 shard rows per core
NBINS = NCORES * WPC
NCHUNK = 4
CSH = SH // NCHUNK          # 3136 rows per (core, class)
CRE = NCORES * CSH          # 25088 rows per gather chunk
TABR = NCHUNK * CRE         # 100352 table rows
GCALL = 32                  # gather groups per dma_gather call

LAST_EXEC_NS = None
LAST_INFO = {}


# ----------------------------------------------------------------------------
# host-side graph prep
# ----------------------------------------------------------------------------
def _prep(src, dst, batch):
    src = np.asarray(src, np.int64)
    dst = np.asarray(dst, np.int64)
    batch = np.asarray(batch, np.int64)

    deg = np.bincount(dst, minlength=N).astype(np.float64) + 1.0
    dinv = 1.0 / np.sqrt(deg)

    cls = np.arange(N, dtype=np.int64) % NCHUNK
    k4 = np.zeros((N, NCHUNK), np.int64)
    np.add.at(k4, (dst, cls[src]), 1)
    ktot = k4.sum(1)

    # --- greedy binning: (core, window) bins, class quotas of 32 ----------
    MARGIN = 12
    capG = {4: 4 * 128 - MARGIN, 5: 5 * 128 - MARGIN, 6: 6 * 128 - MARGIN}
    share = k4.sum(0).max() / max(ktot.sum(), 1)
    Ecore = ktot.sum() / NCORES * 1.004
    Ty = capG[4] / share
    Tz = capG[5] / share
    z = int(np.ceil(max(0.0, (Ecore - WPC * Ty) / (Tz - Ty))))
    z = min(z, WPC)
    wclass = np.array([5] * z + [4] * (WPC - z))

    caps = np.zeros((NBINS, NCHUNK), np.float64)
    for b in range(NBINS):
        caps[b, :] = capG[wclass[b % WPC]]
    rem = caps.copy()
    mu = caps / 128.0
    slots = np.full(NBINS, 128, np.float64)
    clsroom = np.full((NBINS, NCHUNK), 32, np.int64)
    rng = np.random.default_rng(0)
    order = rng.permutation(N)
    bin_of = np.full(N, -1, np.int64)
    for v in order:
        need = k4[v]
        cl = cls[v]
        ok = (rem >= need).all(1) & (slots > 0) & (clsroom[:, cl] > 0)
        if not ok.any():
            ok = (slots > 0) & (clsroom[:, cl] > 0)
            if not ok.any():
                ok = clsroom[:, cl] > 0
        dev = rem - need - (slots[:, None] - 1) * mu
        sc = np.where(ok, (dev * dev).sum(1), np.inf)
        b = int(np.argmin(sc))
        bin_of[v] = b
        rem[b] -= need
        slots[b] -= 1
        clsroom[b, cl] -= 1

    core_of = bin_of // WPC
    w_of = bin_of % WPC

    # slots: class quarters; sloc = running fill per (bin, class)
    sloc_of = np.empty(N, np.int64)
    fill = np.zeros((NBINS, NCHUNK), np.int64)
    sidx = np.argsort(bin_of, kind="stable")
    for v in sidx:
        b, cl = bin_of[v], cls[v]
        sloc_of[v] = fill[b, cl]
        fill[b, cl] += 1
    assert fill.max() <= 32
    s_of = cls * 32 + sloc_of                     # global slot 0..127
    row = cls * CRE + core_of * CSH + sloc_of * 98 + w_of   # table row
    shrow = s_of * WPC + w_of                     # shard-local row

    # unoccupied (c, w, s) slots -> zero rows / pad gather targets
    occ = np.zeros((NCORES, WPC, WIN), bool)
    occ[core_of, w_of, s_of] = True
    uc, uw, us = np.nonzero(~occ)
    unocc_rows = ((us // 32) * CRE + uc * CSH + (us % 32) * 98 + uw)
    pad_iloc = np.zeros(NCHUNK, np.int64)
    for ch in range(NCHUNK):
        cand = unocc_rows[(unocc_rows >= ch * CRE) & (unocc_rows < (ch + 1) * CRE)]
        assert len(cand) > 0, ch
        pad_iloc[ch] = cand[0] % CRE

    # --- per-core edge streams (no self-loops in stream) ------------------
    e_core = core_of[dst]
    e_w = w_of[dst]
    e_slot = s_of[dst].astype(np.float32)
    e_ch = cls[src]
    iloc = (row[src] % CRE).astype(np.int16)
    norm = (dinv[src] * dinv[dst]).astype(np.float32)

    key = (e_core * NCHUNK + e_ch) * WPC + e_w
    nk = NCORES * NCHUNK * WPC
    cnt = np.bincount(key, minlength=nk).reshape(NCORES, NCHUNK, WPC)
    G = np.ceil(cnt.max(axis=0) / 128.0).astype(np.int64)   # [NCHUNK, WPC]
    Gc = G.sum(axis=1)
    GT = int(G.sum())

    # emission order (w, ch, j); chunk-local gather order is (w, j)
    gbase = np.zeros((WPC, NCHUNK), np.int64)
    run = 0
    for w in range(WPC):
        for ch in range(NCHUNK):
            gbase[w, ch] = run
            run += G[ch, w]
    assert run == GT
    cbase = np.zeros((WPC, NCHUNK), np.int64)
    crun = np.zeros(NCHUNK, np.int64)
    for w in range(WPC):
        for ch in range(NCHUNK):
            cbase[w, ch] = crun[ch]
            crun[ch] += G[ch, w]
    assert (crun == Gc).all()

    eorder = np.lexsort((e_ch, e_w, e_core))
    key_s = ((e_core * WPC + e_w) * NCHUNK + e_ch)[eorder]
    iloc_s = iloc[eorder]
    slot_s = e_slot[eorder]
    norm_s = norm[eorder]
    bounds = np.searchsorted(key_s, np.arange(NCORES * WPC * NCHUNK + 1))

    mg = max(int(c) for c in Gc) * 128
    idx_streams = np.zeros((NCORES, NCHUNK, mg), np.int16)
    for ch in range(NCHUNK):
        idx_streams[:, ch, :] = pad_iloc[ch]
    dloc2d = np.full((NCORES, 128, GT), -1.0, np.float32)
    norm2d = np.zeros((NCORES, 128, GT), np.float32)
    for c in range(NCORES):
        for w in range(WPC):
            for ch in range(NCHUNK):
                k = (c * WPC + w) * NCHUNK + ch
                b0, b1 = bounds[k], bounds[k + 1]
                n = b1 - b0
                g = int(G[ch, w])
                assert n <= g * 128
                co = int(cbase[w, ch]) * 128
                idx_streams[c, ch, co:co + n] = iloc_s[b0:b1]
                gg = int(gbase[w, ch])
                sl = np.full(g * 128, -1.0, np.float32)
                nv = np.zeros(g * 128, np.float32)
                sl[:n] = slot_s[b0:b1]
                nv[:n] = norm_s[b0:b1]
                dloc2d[c, :, gg:gg + g] = sl.reshape(g, 128).T
                norm2d[c, :, gg:gg + g] = nv.reshape(g, 128).T

    idx2d = np.zeros((NCORES, NCHUNK, 128, mg // 16), np.int16)
    for c in range(NCORES):
        for ch in range(NCHUNK):
            a = idx_streams[c, ch].reshape(-1, 16).T
            idx2d[c, ch] = np.tile(a, (8, 1))

    # --- per-core aux tensors --------------------------------------------
    nodes = np.arange(N)
    perm = np.full((NCORES, SH), -1, np.int64)    # col i = w*128+s -> node
    perm[core_of, w_of * WIN + s_of] = nodes

    dinv2 = np.zeros((NCORES, WIN, WPC), np.float32)
    dinv2[core_of, s_of, w_of] = (dinv[nodes] ** 2).astype(np.float32)

    counts = np.maximum(np.bincount(batch, minlength=NG), 1).astype(np.float64)
    g2d = np.zeros((NCORES, WIN, WPC * NG), np.float16)
    g2d[core_of, s_of, w_of * NG + batch[nodes]] = (
        1.0 / counts[batch[nodes]]).astype(np.float16)

    return dict(
        G=G, Gc=Gc, GT=GT, gbase=gbase, cbase=cbase,
        idx2d=idx2d, dloc2d=dloc2d, norm2d=norm2d,
        core_of=core_of, w_of=w_of, s_of=s_of, row=row, shrow=shrow,
        perm=perm, dinv2=dinv2, g2d=g2d, counts=counts,
        unocc_rows=np.sort(unocc_rows), pad_iloc=pad_iloc,
    )


# ----------------------------------------------------------------------------
# device program pieces
# ----------------------------------------------------------------------------
def _preload_idx(nc, tc, ctx, idx_aps, sched):
    Gc = sched["Gc"]
    idxc = ctx.enter_context(tc.tile_pool(name="idxc", bufs=1))
    idx_sb = {}
    for ch in range(NCHUNK):
        if Gc[ch] == 0:
            continue
        it = idxc.tile([128, int(Gc[ch]) * 8], I16, tag=f"idx{ch}")
        nc.sync.dma_start(it[:], idx_aps[ch])
        idx_sb[ch] = it
    return idx_sb


def _emit_agg(nc, tc, ctx, tab_ap, dloc_sb, norm_sb, iota_sb, iotacol_sb,
              dinv2_sb, own_sb, bias_sb, hT, sched, winps, idx_sb,
              on_window=None):
    """Aggregate edges + per-window self-loop diag matmul.
    hT[:, w*128:(w+1)*128] = relu(agg_w + bias), feature-major."""
    G, Gc = sched["G"], sched["Gc"]

    gath = {}
    for ch in range(NCHUNK):
        if Gc[ch] == 0:
            continue
        gath[ch] = ctx.enter_context(tc.tile_pool(name=f"gath{ch}", bufs=3))

    tiles = {ch: [] for ch in range(NCHUNK)}
    issued = {ch: 0 for ch in range(NCHUNK)}

    def ensure(ch, upto):
        while issued[ch] <= upto:
            g0 = issued[ch]
            rem = Gc[ch] - g0
            ng = int(min(GCALL if rem > 2 * GCALL else GCALL // 4, rem))
            gt = gath[ch].tile([128, GCALL * F], F16, tag="gt")
            base = ch * CRE
            nc.gpsimd.dma_gather(
                gt[:, :ng * F].rearrange("p (g e) -> p g e", e=F),
                tab_ap[base:base + CRE, :],
                idx_sb[ch][:, g0 * 8:(g0 + ng) * 8], ng * 128, ng * 128, F,
                single_packet=False,
            )
            tiles[ch].append((gt, g0, ng))
            issued[ch] += ng

    ohp = ctx.enter_context(tc.tile_pool(name="ohp", bufs=6))
    gbase, cbase = sched["gbase"], sched["cbase"]
    for w in range(WPC):
        wt = winps.tile([128, 128], F32, tag="win")
        # self-loop diag: oh[s, j] = (j == s) * dinv2[s, w]
        ohs = ohp.tile([128, 128], F16, tag="oh")
        nc.vector.tensor_scalar(
            ohs[:], iota_sb[:], iotacol_sb[:, 0:1], dinv2_sb[:, w:w + 1],
            ALU.is_equal, ALU.mult,
        )
        nc.tensor.matmul(wt[:], own_sb[:, w * F:(w + 1) * F], ohs[:],
                         start=True, stop=False)
        total = int(sum(G[ch, w] for ch in range(NCHUNK)))
        done = 0
        for ch in range(NCHUNK):
            g = int(G[ch, w])
            for j in range(g):
                cg = int(cbase[w, ch]) + j
                ensure(ch, cg)
                gt, g0, ng = next(
                    t for t in tiles[ch] if t[1] <= cg < t[1] + t[2])
                k = cg - g0
                gg = int(gbase[w, ch]) + j
                oh = ohp.tile([128, 128], F16, tag="oh")
                nc.vector.tensor_scalar(
                    oh[:], iota_sb[:], dloc_sb[:, gg:gg + 1],
                    norm_sb[:, gg:gg + 1], ALU.is_equal, ALU.mult,
                )
                done += 1
                nc.tensor.matmul(
                    wt[:], gt[:, k * F:(k + 1) * F], oh[:],
                    start=False, stop=(done == total),
                )
        nc.scalar.activation(hT[:, w * F:(w + 1) * F], wt[:], AF.Relu,
                             bias=bias_sb[:, 0:1])
        if on_window is not None:
            on_window(w)


# ----------------------------------------------------------------------------
# builders
# ----------------------------------------------------------------------------
def _build_launch1():
    """Sharded dense D1: xTc [128, SH] -> g1s [SH, 128] (rows s*98+w)."""
    nc = bacc.Bacc("TRN2", target_bir_lowering=False, debug=False,
                   num_devices=NCORES)
    xT = nc.dram_tensor("xT", [128, SH], F16, kind="ExternalInput")
    w1 = nc.dram_tensor("w1", [128, 128], F16, kind="ExternalInput")
    wc1 = nc.dram_tensor("wc1", [128, 128], F16, kind="ExternalInput")
    b1 = nc.dram_tensor("b1", [128, 1], F32, kind="ExternalInput")
    g1s = nc.dram_tensor("g1s", [SH, F], F16, kind="ExternalOutput")

    with tile.TileContext(nc) as tc, contextlib.ExitStack() as ctx:
        const = ctx.enter_context(tc.tile_pool(name="const", bufs=1))
        w1_sb = const.tile([128, 128], F16, tag="w1")
        nc.sync.dma_start(w1_sb[:], w1.ap())
        wc1_sb = const.tile([128, 128], F16, tag="wc1")
        nc.sync.dma_start(wc1_sb[:], wc1.ap())
        b1_sb = const.tile([128, 1], F32, tag="b1")
        nc.sync.dma_start(b1_sb[:], b1.ap())

        mm1ps = ctx.enter_context(tc.tile_pool(name="mm1ps", bufs=2,
                                               space="PSUM"))
        tabps = ctx.enter_context(tc.tile_pool(name="tabps", bufs=2,
                                               space="PSUM"))
        hpool = ctx.enter_context(tc.tile_pool(name="hpool", bufs=3))
        stpool = ctx.enter_context(tc.tile_pool(name="stpool", bufs=3))
        xin = ctx.enter_context(tc.tile_pool(name="xin", bufs=3))

        g1s_pm = g1s.ap().rearrange("(s u) f -> s (u f)", s=128)
        xt_cache = {}

        def src1(u512):
            blk = u512 // 4
            if blk not in xt_cache:
                t = xin.tile([128, 2048], F16, tag="xt")
                cw = min(2048, SH - blk * 2048)
                nc.scalar.dma_start(t[:, :cw], xT.ap()[:, blk * 2048:blk * 2048 + cw])
                xt_cache.clear()
                xt_cache[blk] = t
            return xt_cache[blk]

        NT = (SH + 511) // 512          # 25 tiles, last = 256 cols
        st = None
        for u512 in range(NT):
            c0 = u512 * 512
            cw = min(512, SH - c0)
            xt = src1(u512)
            xs = xt[:, (u512 % 4) * 512:(u512 % 4) * 512 + cw]
            p1 = mm1ps.tile([128, 512], F32, tag="p1")
            nc.tensor.matmul(p1[:, :cw], w1_sb[:], xs, start=True, stop=True)
            h1 = hpool.tile([128, 512], F16, tag="h1")
            nc.scalar.activation(h1[:, :cw], p1[:, :cw], AF.Relu,
                                 bias=b1_sb[:, 0:1])
            q8 = u512 % 2
            if q8 == 0:
                st = stpool.tile([128, 1024], F16, tag="st")
                tp = tabps.tile([128, 1024], F32, tag="tp")
                _build_launch1.tp = tp
            tp = _build_launch1.tp
            for q in range(cw // 128):
                nc.tensor.matmul(
                    tp[:, (q8 * 4 + q) * 128:(q8 * 4 + q + 1) * 128],
                    h1[:, q * 128:(q + 1) * 128], wc1_sb[:],
                    start=True, stop=True,
                )
            if q8 == 1 or u512 == NT - 1:
                nu = q8 * 4 + cw // 128
                nc.vector.tensor_copy(st[:, :nu * 128], tp[:, :nu * 128])
                u0 = (u512 // 2) * 8
                nc.sync.dma_start(g1s_pm[:, u0 * 128:(u0 + nu) * 128],
                                  st[:, :nu * 128])

    nc.compile()
    return nc


def _build_launch2(prep):
    """Aggregate conv1 + fused D2 -> g2s shard rows."""
    nc = bacc.Bacc("TRN2", target_bir_lowering=False, debug=False,
                   num_devices=NCORES)
    GT = prep["GT"]
    Gc = prep["Gc"]

    tab1 = nc.dram_tensor("tab1", [TABR, F], F16, kind="ExternalInput")
    own1 = nc.dram_tensor("own1", [128, WPC * F], F16, kind="ExternalInput")
    idxs = [nc.dram_tensor("idx%d" % ch, [128, int(Gc[ch]) * 8], I16,
                           kind="ExternalInput") for ch in range(NCHUNK)]
    dloc = nc.dram_tensor("dloc", [128, GT], F32, kind="ExternalInput")
    norm = nc.dram_tensor("norm", [128, GT], F32, kind="ExternalInput")
    dinv2 = nc.dram_tensor("dinv2", [128, WPC], F32, kind="ExternalInput")
    iota = nc.dram_tensor("iota", [128, 128], F16, kind="ExternalInput")
    iotac = nc.dram_tensor("iotac", [128, 1], F32, kind="ExternalInput")
    wfc2 = nc.dram_tensor("wfc2", [128, 128], F16, kind="ExternalInput")
    wc2 = nc.dram_tensor("wc2", [128, 128], F16, kind="ExternalInput")
    bc1 = nc.dram_tensor("bc1", [128, 1], F32, kind="ExternalInput")
    bfc2 = nc.dram_tensor("bfc2", [128, 1], F32, kind="ExternalInput")
    g2s = nc.dram_tensor("g2s", [SH, F], F16, kind="ExternalOutput")

    with tile.TileContext(nc) as tc, contextlib.ExitStack() as ctx:
        idx_sb = _preload_idx(nc, tc, ctx, [a.ap() for a in idxs], prep)

        const = ctx.enter_context(tc.tile_pool(name="const", bufs=1))
        big = ctx.enter_context(tc.tile_pool(name="big", bufs=1))
        _n = [0]

        def ld(ap, shape, dtype):
            _n[0] += 1
            t = const.tile(shape, dtype, tag="c%d" % _n[0])
            nc.sync.dma_start(t[:], ap)
            return t

        dloc_sb = ld(dloc.ap(), [128, GT], F32)
        norm_sb = ld(norm.ap(), [128, GT], F32)
        own_sb = big.tile([128, WPC * F], F16, tag="own")
        nc.scalar.dma_start(own_sb[:], own1.ap())
        dinv2_sb = ld(dinv2.ap(), [128, WPC], F32)
        iota_sb = ld(iota.ap(), [128, 128], F16)
        iotac_sb = ld(iotac.ap(), [128, 1], F32)
        wfc2_sb = ld(wfc2.ap(), [128, 128], F16)
        wc2_sb = ld(wc2.ap(), [128, 128], F16)
        bc1_sb = ld(bc1.ap(), [128, 1], F32)
        bfc2_sb = ld(bfc2.ap(), [128, 1], F32)
        h2T = big.tile([128, SH], F16, tag="h2T")

        winps = ctx.enter_context(tc.tile_pool(name="winps", bufs=2,
                                               space="PSUM"))
        mm1ps = ctx.enter_context(tc.tile_pool(name="mm1ps", bufs=2,
                                               space="PSUM"))
        tabps = ctx.enter_context(tc.tile_pool(name="tabps", bufs=2,
                                               space="PSUM"))
        hpool = ctx.enter_context(tc.tile_pool(name="hpool", bufs=3))
        stpool = ctx.enter_context(tc.tile_pool(name="stpool", bufs=3))

        g2s_pm = g2s.ap().rearrange("(s u) f -> s (u f)", s=128)
        d2state = {}

        def d2_tile(w):
            if w % 4 != 3 and w != WPC - 1:
                return
            u512 = w // 4
            c0 = u512 * 512
            cw = min(512, SH - c0)
            p1 = mm1ps.tile([128, 512], F32, tag="p1")
            nc.tensor.matmul(p1[:, :cw], wfc2_sb[:], h2T[:, c0:c0 + cw],
                             start=True, stop=True)
            h1 = hpool.tile([128, 512], F16, tag="h1")
            nc.scalar.activation(h1[:, :cw], p1[:, :cw], AF.Relu,
                                 bias=bfc2_sb[:, 0:1])
            q8 = u512 % 2
            if q8 == 0:
                st_new = stpool.tile([128, 1024], F16, tag="st")
                tp_new = tabps.tile([128, 1024], F32, tag="tp")
                d2state["st"] = st_new
                d2state["tp"] = tp_new
            tp2 = d2state["tp"]
            for q in range(cw // 128):
                nc.tensor.matmul(
                    tp2[:, (q8 * 4 + q) * 128:(q8 * 4 + q + 1) * 128],
                    h1[:, q * 128:(q + 1) * 128], wc2_sb[:],
                    start=True, stop=True,
                )
            if q8 == 1 or u512 == (SH - 1) // 512:
                nu = q8 * 4 + cw // 128
                st = d2state["st"]
                nc.vector.tensor_copy(st[:, :nu * 128], tp2[:, :nu * 128])
                u0 = (u512 // 2) * 8
                nc.sync.dma_start(g2s_pm[:, u0 * 128:(u0 + nu) * 128],
                                  st[:, :nu * 128])

        _emit_agg(nc, tc, ctx, tab1.ap(), dloc_sb, norm_sb, iota_sb,
                  iotac_sb, dinv2_sb, own_sb, bc1_sb, h2T, prep, winps,
                  idx_sb, on_window=d2_tile)

    nc.compile()
    return nc


def _build_launch3(prep):
    """Aggregate conv2 + D3 + graph-pool partials."""
    nc = bacc.Bacc("TRN2", target_bir_lowering=False, debug=False,
                   num_devices=NCORES)
    GT = prep["GT"]
    Gc = prep["Gc"]

    tab2 = nc.dram_tensor("tab2", [TABR, F], F16, kind="ExternalInput")
    own2 = nc.dram_tensor("own2", [128, WPC * F], F16, kind="ExternalInput")
    idxs = [nc.dram_tensor("idx%d" % ch, [128, int(Gc[ch]) * 8], I16,
                           kind="ExternalInput") for ch in range(NCHUNK)]
    dloc = nc.dram_tensor("dloc", [128, GT], F32, kind="ExternalInput")
    norm = nc.dram_tensor("norm", [128, GT], F32, kind="ExternalInput")
    dinv2 = nc.dram_tensor("dinv2", [128, WPC], F32, kind="ExternalInput")
    iota = nc.dram_tensor("iota", [128, 128], F16, kind="ExternalInput")
    iotac = nc.dram_tensor("iotac", [128, 1], F32, kind="ExternalInput")
    wfc = nc.dram_tensor("wfc", [128, NOUT], F16, kind="ExternalInput")
    bc2 = nc.dram_tensor("bc2", [128, 1], F32, kind="ExternalInput")
    g2d = nc.dram_tensor("g2d", [128, WPC * NG], F16, kind="ExternalInput")
    pool = nc.dram_tensor("pool", [NG, NOUT], F32, kind="ExternalOutput")

    with tile.TileContext(nc) as tc, contextlib.ExitStack() as ctx:
        idx_sb = _preload_idx(nc, tc, ctx, [a.ap() for a in idxs], prep)

        const = ctx.enter_context(tc.tile_pool(name="const", bufs=1))
        big = ctx.enter_context(tc.tile_pool(name="big", bufs=1))
        _n = [0]

        def ld(ap, shape, dtype):
            _n[0] += 1
            t = const.tile(shape, dtype, tag="c%d" % _n[0])
            nc.sync.dma_start(t[:], ap)
            return t

        dloc_sb = ld(dloc.ap(), [128, GT], F32)
        norm_sb = ld(norm.ap(), [128, GT], F32)
        own_sb = big.tile([128, WPC * F], F16, tag="own")
        nc.scalar.dma_start(own_sb[:], own2.ap())
        dinv2_sb = ld(dinv2.ap(), [128, WPC], F32)
        iota_sb = ld(iota.ap(), [128, 128], F16)
        iotac_sb = ld(iotac.ap(), [128, 1], F32)
        wfc_sb = ld(wfc.ap(), [128, NOUT], F16)
        bc2_sb = ld(bc2.ap(), [128, 1], F32)
        g2d_sb = ld(g2d.ap(), [128, WPC * NG], F16)
        h4T = big.tile([128, SH], F16, tag="h4T")

        winps = ctx.enter_context(tc.tile_pool(name="winps", bufs=2,
                                               space="PSUM"))
        psd = ctx.enter_context(tc.tile_pool(name="psd", bufs=3, space="PSUM"))
        osb = ctx.enter_context(tc.tile_pool(name="osb", bufs=4))
        psp = ctx.enter_context(tc.tile_pool(name="psp", bufs=1, space="PSUM"))
        poolps = psp.tile([NG, NOUT], F32)

        def d3_win(w):
            pd = psd.tile([128, NOUT], F32, tag="pd")
            nc.tensor.matmul(pd[:], h4T[:, w * F:(w + 1) * F], wfc_sb[:],
                             start=True, stop=True)
            ot = osb.tile([128, NOUT], F16, tag="ot")
            nc.scalar.activation(ot[:], pd[:], AF.Copy)
            nc.tensor.matmul(poolps[:], g2d_sb[:, w * NG:(w + 1) * NG],
                             ot[:], start=(w == 0), stop=(w == WPC - 1),
                             skip_group_check=True)

        _emit_agg(nc, tc, ctx, tab2.ap(), dloc_sb, norm_sb, iota_sb,
                  iotac_sb, dinv2_sb, own_sb, bc2_sb, h4T, prep, winps,
                  idx_sb, on_window=d3_win)

        pres = osb.tile([NG, NOUT], F32, tag="pres")
        nc.vector.tensor_copy(pres[:], poolps[:])
        nc.sync.dma_start(pool.ap(), pres[:])

    nc.compile()
    return nc


def _np16(x):
    return np.ascontiguousarray(x, np.float16)


def _tl_ns(nc):
    from concourse.timeline_sim import TimelineSim
    tl = TimelineSim(nc, trace=False)
    tl.simulate()
    return int(tl.time)


def kernel(x, src, dst, batch, W_fc1, b_fc1, W_c1, b_c1, W_fc2, b_fc2, W_c2,
           b_c2, W_fc, b_fc):
    global LAST_EXEC_NS, LAST_INFO
    x = np.asarray(x, np.float32)
    prep = _prep(src, dst, batch)
    trace = os.environ.get("KERNEL_TRACE", "0") == "1"
    timing = os.environ.get("KERNEL_TIME", "0") == "1"

    col = lambda b: np.ascontiguousarray(
        np.asarray(b, np.float32).reshape(-1, 1))
    iota = np.tile(np.arange(128, dtype=np.float16), (128, 1))
    iotac = np.arange(128, dtype=np.float32).reshape(128, 1)

    # ---- launch 1: sharded dense -> g1s shards --------------------------
    nc1 = _build_launch1()
    in_maps1 = []
    for c in range(NCORES):
        pm = prep["perm"][c]
        xTc = np.zeros((SH, F), np.float16)
        m = pm >= 0
        xTc[m] = x[pm[m]].astype(np.float16)
        in_maps1.append({
            "xT": np.ascontiguousarray(xTc.T), "w1": _np16(W_fc1),
            "wc1": _np16(W_c1), "b1": col(b_fc1),
        })
    r1 = run_bass_kernel_spmd(nc1, in_maps1, core_ids=list(range(NCORES)),
                              trace=trace)
    t1 = r1.exec_time_ns or (_tl_ns(nc1) if timing else None)

    # host: assemble table1 + own views
    g1 = [np.asarray(r1.results[c]["g1s"]) for c in range(NCORES)]
    tab1 = np.zeros((TABR, F), np.float16)
    t1v = tab1.reshape(NCHUNK, NCORES, CSH, F)
    for c in range(NCORES):
        t1v[:, c] = g1[c].reshape(NCHUNK, CSH, F)
    tab1[prep["unocc_rows"]] = 0.0

    def own_view(gs):
        return np.ascontiguousarray(gs.reshape(128, WPC * F))

    def im_agg(c, tabname, tabv, ownv, extra):
        im = {
            tabname: tabv, "dloc": prep["dloc2d"][c], "norm": prep["norm2d"][c],
            "dinv2": np.ascontiguousarray(prep["dinv2"][c]),
            "iota": iota, "iotac": iotac,
        }
        for ch in range(NCHUNK):
            gc = int(prep["Gc"][ch]) * 8
            im["idx%d" % ch] = np.ascontiguousarray(
                prep["idx2d"][c, ch][:, :gc])
        im.update(extra)
        im["own1" if tabname == "tab1" else "own2"] = ownv
        return im

    # ---- launch 2: agg conv1 + D2 -> g2s shards -------------------------
    nc2 = _build_launch2(prep)
    in_maps2 = [
        im_agg(c, "tab1", tab1, own_view(g1[c]), {
            "wfc2": _np16(W_fc2), "wc2": _np16(W_c2),
            "bc1": col(b_c1), "bfc2": col(b_fc2),
        }) for c in range(NCORES)
    ]
    r2 = run_bass_kernel_spmd(nc2, in_maps2, core_ids=list(range(NCORES)),
                              trace=trace)
    t2 = r2.exec_time_ns or (_tl_ns(nc2) if timing else None)

    g2 = [np.asarray(r2.results[c]["g2s"]) for c in range(NCORES)]
    tab2 = np.zeros((TABR, F), np.float16)
    t2v = tab2.reshape(NCHUNK, NCORES, CSH, F)
    for c in range(NCORES):
        t2v[:, c] = g2[c].reshape(NCHUNK, CSH, F)
    tab2[prep["unocc_rows"]] = 0.0

    # ---- launch 3: agg conv2 + D3 + pool --------------------------------
    nc3 = _build_launch3(prep)
    in_maps3 = [
        im_agg(c, "tab2", tab2, own_view(g2[c]), {
            "wfc": _np16(W_fc), "bc2": col(b_c2),
            "g2d": np.ascontiguousarray(prep["g2d"][c]),
        }) for c in range(NCORES)
    ]
    r3 = run_bass_kernel_spmd(nc3, in_maps3, core_ids=list(range(NCORES)),
                              trace=trace)
    t3 = r3.exec_time_ns or (_tl_ns(nc3) if timing else None)

    out = np.zeros((NG, NOUT), np.float64)
    for c in range(NCORES):
        out += np.asarray(r3.results[c]["pool"]).astype(np.float64)
    out = out + np.asarray(b_fc, np.float64)[None, :]

    LAST_EXEC_NS = (t1 or 0) + (t2 or 0) + (t3 or 0)
    LAST_INFO = {"t1": t1, "t2": t2, "t3": t3, "GT": prep["GT"]}
    return out.astype(np.float32)


# revision 6
# speedup vs baseline: 1.2686x; 1.0186x over previous
"""2-layer GCN + dense layers + mean-pool on 8 trn2 NeuronCores (Bass/Tile).

v3 design (3 launches, sharded dense, self-loops via local diag matmuls).

GCNConv out = D^-1/2 (A+I) D^-1/2 (h W) + b factorizes as
  table[v]  = (h W)[v]                      (unscaled, per node)
  agg[d]    = sum_{e: dst=d} norm_e * table[src_e] + dinv_d^2 * table[d]
  h2[d]     = relu(agg[d] + b)
Self-loop terms never enter the gather stream: each core keeps its own dst
shard's table rows (slot-major, contiguous) in SBUF and adds them with one
matmul per window against a DVE-built diagonal (values dinv_d^2).

Node placement: greedy binning assigns each node to a (core, window) bin
(128 slots each) with per-chunk caps, where chunk class = node_id % 4 and
slots are class quarters (slot = class*32 + sloc).  Table row of node v:
  row(v) = class*25088 + core*3136 + sloc*98 + w   (= core shard row s*98+w)
so each core's dense output shard [12544, 128] is contiguous, gather chunks
(int16 idx) are fixed row ranges, and host-side assembly is pure reshape.

Launch 1: per-core dense D1 (x shard -> relu(xW1+b1) Wc1) -> shard rows.
Host: assemble table1, build own-shard views.  Launch 2: aggregate conv1
(dma_gather per 128-edge group + one-hot matmul, feature-major windows),
fused D2 -> table2 shard rows.  Host: assemble table2.  Launch 3: aggregate
conv2 + D3 + graph-pool partials; host sums partials + b_fc.

Both conv layers share one gather schedule (same graph, same row map).
"""

import os
import sys

sys.path.insert(0, "/opt/trn_rl_repo")

import contextlib

import numpy as np

import concourse.bass as bass
import concourse.tile as tile
from concourse import bacc, mybir
from concourse.bass_utils import run_bass_kernel_spmd

F32 = mybir.dt.float32
F16 = mybir.dt.float16
I16 = mybir.dt.int16
FP8 = mybir.dt.float8e4
AF = mybir.ActivationFunctionType
ALU = mybir.AluOpType

N = 100000
F = 128
NOUT = 64
NG = 64
NCORES = 8
WPC = 98                    # windows per core
WIN = 128
SH = WPC * WIN              # 12544 shard rows per core
NBINS = NCORES * WPC
NCHUNK = 4
CSH = SH // NCHUNK          # 3136 rows per (core, class)
CRE = NCORES * CSH          # 25088 rows per gather chunk
TABR = NCHUNK * CRE         # 100352 table rows
GCALL = 32                  # gather groups per dma_gather call

LAST_EXEC_NS = None
LAST_INFO = {}


# ----------------------------------------------------------------------------
# host-side graph prep
# ----------------------------------------------------------------------------
def _prep(src, dst, batch):
    src = np.asarray(src, np.int64)
    dst = np.asarray(dst, np.int64)
    batch = np.asarray(batch, np.int64)

    deg = np.bincount(dst, minlength=N).astype(np.float64) + 1.0
    dinv = 1.0 / np.sqrt(deg)

    cls = np.arange(N, dtype=np.int64) % NCHUNK
    k4 = np.zeros((N, NCHUNK), np.int64)
    np.add.at(k4, (dst, cls[src]), 1)
    ktot = k4.sum(1)

    # --- greedy binning: (core, window) bins, class quotas of 32 ----------
    MARGIN = 12
    capG = {4: 4 * 128 - MARGIN, 5: 5 * 128 - MARGIN, 6: 6 * 128 - MARGIN}
    share = k4.sum(0).max() / max(ktot.sum(), 1)
    Ecore = ktot.sum() / NCORES * 1.004
    Ty = capG[4] / share
    Tz = capG[5] / share
    z = int(np.ceil(max(0.0, (Ecore - WPC * Ty) / (Tz - Ty))))
    z = min(z, WPC)
    wclass = np.array([5] * z + [4] * (WPC - z))

    caps = np.zeros((NBINS, NCHUNK), np.float64)
    for b in range(NBINS):
        caps[b, :] = capG[wclass[b % WPC]]
    rem = caps.copy()
    mu = caps / 128.0
    slots = np.full(NBINS, 128, np.float64)
    clsroom = np.full((NBINS, NCHUNK), 32, np.int64)
    rng = np.random.default_rng(0)
    order = rng.permutation(N)
    bin_of = np.full(N, -1, np.int64)
    for v in order:
        need = k4[v]
        cl = cls[v]
        ok = (rem >= need).all(1) & (slots > 0) & (clsroom[:, cl] > 0)
        if not ok.any():
            ok = (slots > 0) & (clsroom[:, cl] > 0)
            if not ok.any():
                ok = clsroom[:, cl] > 0
        dev = rem - need - (slots[:, None] - 1) * mu
        sc = np.where(ok, (dev * dev).sum(1), np.inf)
        b = int(np.argmin(sc))
        bin_of[v] = b
        rem[b] -= need
        slots[b] -= 1
        clsroom[b, cl] -= 1

    core_of = bin_of // WPC
    w_of = bin_of % WPC

    # slots: class quarters; sloc = running fill per (bin, class)
    sloc_of = np.empty(N, np.int64)
    fill = np.zeros((NBINS, NCHUNK), np.int64)
    sidx = np.argsort(bin_of, kind="stable")
    for v in sidx:
        b, cl = bin_of[v], cls[v]
        sloc_of[v] = fill[b, cl]
        fill[b, cl] += 1
    assert fill.max() <= 32
    s_of = cls * 32 + sloc_of                     # global slot 0..127
    row = cls * CRE + core_of * CSH + sloc_of * 98 + w_of   # table row
    shrow = s_of * WPC + w_of                     # shard-local row

    # unoccupied (c, w, s) slots -> zero rows / pad gather targets
    occ = np.zeros((NCORES, WPC, WIN), bool)
    occ[core_of, w_of, s_of] = True
    uc, uw, us = np.nonzero(~occ)
    unocc_rows = ((us // 32) * CRE + uc * CSH + (us % 32) * 98 + uw)
    pad_iloc = np.zeros(NCHUNK, np.int64)
    for ch in range(NCHUNK):
        cand = unocc_rows[(unocc_rows >= ch * CRE) & (unocc_rows < (ch + 1) * CRE)]
        assert len(cand) > 0, ch
        pad_iloc[ch] = cand[0] % CRE

    # --- per-core edge streams (no self-loops in stream) ------------------
    e_core = core_of[dst]
    e_w = w_of[dst]
    e_slot = s_of[dst].astype(np.float32)
    e_ch = cls[src]
    iloc = (row[src] % CRE).astype(np.int16)
    norm = (dinv[src] * dinv[dst]).astype(np.float32)

    key = (e_core * NCHUNK + e_ch) * WPC + e_w
    nk = NCORES * NCHUNK * WPC
    cnt = np.bincount(key, minlength=nk).reshape(NCORES, NCHUNK, WPC)
    G = np.ceil(cnt.max(axis=0) / 128.0).astype(np.int64)   # [NCHUNK, WPC]
    Gc = G.sum(axis=1)
    GT = int(G.sum())

    # emission order (w, ch, j); chunk-local gather order is (w, j)
    gbase = np.zeros((WPC, NCHUNK), np.int64)
    run = 0
    for w in range(WPC):
        for ch in range(NCHUNK):
            gbase[w, ch] = run
            run += G[ch, w]
    assert run == GT
    cbase = np.zeros((WPC, NCHUNK), np.int64)
    crun = np.zeros(NCHUNK, np.int64)
    for w in range(WPC):
        for ch in range(NCHUNK):
            cbase[w, ch] = crun[ch]
            crun[ch] += G[ch, w]
    assert (crun == Gc).all()

    eorder = np.lexsort((e_ch, e_w, e_core))
    key_s = ((e_core * WPC + e_w) * NCHUNK + e_ch)[eorder]
    iloc_s = iloc[eorder]
    slot_s = e_slot[eorder]
    norm_s = norm[eorder]
    bounds = np.searchsorted(key_s, np.arange(NCORES * WPC * NCHUNK + 1))

    mg = max(int(c) for c in Gc) * 128
    idx_streams = np.zeros((NCORES, NCHUNK, mg), np.int16)
    for ch in range(NCHUNK):
        idx_streams[:, ch, :] = pad_iloc[ch]
    dloc2d = np.full((NCORES, 128, GT), -1.0, np.float32)
    norm2d = np.zeros((NCORES, 128, GT), np.float32)
    for c in range(NCORES):
        for w in range(WPC):
            for ch in range(NCHUNK):
                k = (c * WPC + w) * NCHUNK + ch
                b0, b1 = bounds[k], bounds[k + 1]
                n = b1 - b0
                g = int(G[ch, w])
                assert n <= g * 128
                co = int(cbase[w, ch]) * 128
                idx_streams[c, ch, co:co + n] = iloc_s[b0:b1]
                gg = int(gbase[w, ch])
                sl = np.full(g * 128, -1.0, np.float32)
                nv = np.zeros(g * 128, np.float32)
                sl[:n] = slot_s[b0:b1]
                nv[:n] = norm_s[b0:b1]
                dloc2d[c, :, gg:gg + g] = sl.reshape(g, 128).T
                norm2d[c, :, gg:gg + g] = nv.reshape(g, 128).T

    idx2d = np.zeros((NCORES, NCHUNK, 128, mg // 16), np.int16)
    for c in range(NCORES):
        for ch in range(NCHUNK):
            a = idx_streams[c, ch].reshape(-1, 16).T
            idx2d[c, ch] = np.tile(a, (8, 1))

    # --- per-core aux tensors --------------------------------------------
    nodes = np.arange(N)
    perm = np.full((NCORES, SH), -1, np.int64)    # col i = w*128+s -> node
    perm[core_of, w_of * WIN + s_of] = nodes

    dinv2 = np.zeros((NCORES, WIN, WPC), np.float32)
    dinv2[core_of, s_of, w_of] = (dinv[nodes] ** 2).astype(np.float32)

    counts = np.maximum(np.bincount(batch, minlength=NG), 1).astype(np.float64)
    g2d = np.zeros((NCORES, WIN, WPC * NG), np.float16)
    g2d[core_of, s_of, w_of * NG + batch[nodes]] = (
        1.0 / counts[batch[nodes]]).astype(np.float16)

    return dict(
        G=G, Gc=Gc, GT=GT, gbase=gbase, cbase=cbase,
        idx2d=idx2d, dloc2d=dloc2d, norm2d=norm2d,
        core_of=core_of, w_of=w_of, s_of=s_of, row=row, shrow=shrow,
        perm=perm, dinv2=dinv2, g2d=g2d, counts=counts,
        unocc_rows=np.sort(unocc_rows), pad_iloc=pad_iloc,
    )


# ----------------------------------------------------------------------------
# device program pieces
# ----------------------------------------------------------------------------
def _preload_idx(nc, tc, ctx, idx_aps, sched):
    Gc = sched["Gc"]
    idxc = ctx.enter_context(tc.tile_pool(name="idxc", bufs=1))
    idx_sb = {}
    for ch in range(NCHUNK):
        if Gc[ch] == 0:
            continue
        it = idxc.tile([128, int(Gc[ch]) * 8], I16, tag=f"idx{ch}")
        nc.sync.dma_start(it[:], idx_aps[ch])
        idx_sb[ch] = it
    return idx_sb


def _emit_agg(nc, tc, ctx, tab_ap, dloc_sb, norm_sb, iota_sb, iotacol_sb,
              dinv2_sb, own_sb, bias_sb, hT, sched, winps, idx_sb,
              on_window=None):
    """Aggregate edges + per-window self-loop diag matmul.
    hT[:, w*128:(w+1)*128] = relu(agg_w + bias), feature-major."""
    G, Gc = sched["G"], sched["Gc"]

    gath = {}
    for ch in range(NCHUNK):
        if Gc[ch] == 0:
            continue
        gath[ch] = ctx.enter_context(tc.tile_pool(name=f"gath{ch}", bufs=3))

    tiles = {ch: [] for ch in range(NCHUNK)}
    issued = {ch: 0 for ch in range(NCHUNK)}

    def ensure(ch, upto):
        while issued[ch] <= upto:
            g0 = issued[ch]
            rem = Gc[ch] - g0
            ng = int(min(GCALL if rem > 2 * GCALL else GCALL // 2, rem))
            gt = gath[ch].tile([128, GCALL * F], F16, tag="gt")
            base = ch * CRE
            nc.gpsimd.dma_gather(
                gt[:, :ng * F].rearrange("p (g e) -> p g e", e=F),
                tab_ap[base:base + CRE, :],
                idx_sb[ch][:, g0 * 8:(g0 + ng) * 8], ng * 128, ng * 128, F,
                single_packet=False,
            )
            tiles[ch].append((gt, g0, ng))
            issued[ch] += ng

    ohp = ctx.enter_context(tc.tile_pool(name="ohp", bufs=6))
    gbase, cbase = sched["gbase"], sched["cbase"]
    for w in range(WPC):
        wt = winps.tile([128, 128], F32, tag="win")
        # self-loop diag: oh[s, j] = (j == s) * dinv2[s, w]
        ohs = ohp.tile([128, 128], F16, tag="oh")
        nc.vector.tensor_scalar(
            ohs[:], iota_sb[:], iotacol_sb[:, 0:1], dinv2_sb[:, w:w + 1],
            ALU.is_equal, ALU.mult,
        )
        nc.tensor.matmul(wt[:], own_sb[:, w * F:(w + 1) * F], ohs[:],
                         start=True, stop=False)
        total = int(sum(G[ch, w] for ch in range(NCHUNK)))
        done = 0
        for ch in range(NCHUNK):
            g = int(G[ch, w])
            for j in range(g):
                cg = int(cbase[w, ch]) + j
                ensure(ch, cg)
                gt, g0, ng = next(
                    t for t in tiles[ch] if t[1] <= cg < t[1] + t[2])
                k = cg - g0
                gg = int(gbase[w, ch]) + j
                oh = ohp.tile([128, 128], F16, tag="oh")
                nc.vector.tensor_scalar(
                    oh[:], iota_sb[:], dloc_sb[:, gg:gg + 1],
                    norm_sb[:, gg:gg + 1], ALU.is_equal, ALU.mult,
                )
                done += 1
                nc.tensor.matmul(
                    wt[:], gt[:, k * F:(k + 1) * F], oh[:],
                    start=False, stop=(done == total),
                )
        nc.scalar.activation(hT[:, w * F:(w + 1) * F], wt[:], AF.Relu,
                             bias=bias_sb[:, 0:1])
        if on_window is not None:
            on_window(w)


# ----------------------------------------------------------------------------
# builders
# ----------------------------------------------------------------------------
def _build_launch1():
    """Sharded dense D1: xTc [128, SH] -> g1s [SH, 128] (rows s*98+w)."""
    nc = bacc.Bacc("TRN2", target_bir_lowering=False, debug=False,
                   num_devices=NCORES)
    xT = nc.dram_tensor("xT", [128, SH], FP8, kind="ExternalInput")
    w1 = nc.dram_tensor("w1", [128, 128], F16, kind="ExternalInput")
    wc1 = nc.dram_tensor("wc1", [128, 128], F16, kind="ExternalInput")
    b1 = nc.dram_tensor("b1", [128, 1], F32, kind="ExternalInput")
    g1s = nc.dram_tensor("g1s", [SH, F], FP8, kind="ExternalOutput")

    with tile.TileContext(nc) as tc, contextlib.ExitStack() as ctx:
        const = ctx.enter_context(tc.tile_pool(name="const", bufs=1))
        w1_sb = const.tile([128, 128], F16, tag="w1")
        nc.sync.dma_start(w1_sb[:], w1.ap())
        wc1_sb = const.tile([128, 128], F16, tag="wc1")
        nc.sync.dma_start(wc1_sb[:], wc1.ap())
        b1_sb = const.tile([128, 1], F32, tag="b1")
        nc.sync.dma_start(b1_sb[:], b1.ap())

        mm1ps = ctx.enter_context(tc.tile_pool(name="mm1ps", bufs=2,
                                               space="PSUM"))
        tabps = ctx.enter_context(tc.tile_pool(name="tabps", bufs=2,
                                               space="PSUM"))
        hpool = ctx.enter_context(tc.tile_pool(name="hpool", bufs=3))
        stpool = ctx.enter_context(tc.tile_pool(name="stpool", bufs=3))
        xin = ctx.enter_context(tc.tile_pool(name="xin", bufs=3))

        g1s_pm = g1s.ap().rearrange("(s u) f -> s (u f)", s=128)
        xt_cache = {}

        def src1(u512):
            blk = u512 // 4
            if blk not in xt_cache:
                t = xin.tile([128, 2048], FP8, tag="xt")
                cw = min(2048, SH - blk * 2048)
                nc.scalar.dma_start(t[:, :cw], xT.ap()[:, blk * 2048:blk * 2048 + cw])
                xt_cache.clear()
                xt_cache[blk] = t
            return xt_cache[blk]

        NT = (SH + 511) // 512          # 25 tiles, last = 256 cols
        st = None
        for u512 in range(NT):
            c0 = u512 * 512
            cw = min(512, SH - c0)
            xt = src1(u512)
            xs = xt[:, (u512 % 4) * 512:(u512 % 4) * 512 + cw]
            p1 = mm1ps.tile([128, 512], F32, tag="p1")
            nc.tensor.matmul(p1[:, :cw], w1_sb[:], xs, start=True, stop=True)
            h1 = hpool.tile([128, 512], F16, tag="h1")
            nc.scalar.activation(h1[:, :cw], p1[:, :cw], AF.Relu,
                                 bias=b1_sb[:, 0:1])
            q8 = u512 % 2
            if q8 == 0:
                st = stpool.tile([128, 1024], FP8, tag="st")
                tp = tabps.tile([128, 1024], F32, tag="tp")
                _build_launch1.tp = tp
            tp = _build_launch1.tp
            for q in range(cw // 128):
                nc.tensor.matmul(
                    tp[:, (q8 * 4 + q) * 128:(q8 * 4 + q + 1) * 128],
                    h1[:, q * 128:(q + 1) * 128], wc1_sb[:],
                    start=True, stop=True,
                )
            if q8 == 1 or u512 == NT - 1:
                nu = q8 * 4 + cw // 128
                nc.vector.tensor_copy(st[:, :nu * 128], tp[:, :nu * 128])
                u0 = (u512 // 2) * 8
                nc.sync.dma_start(g1s_pm[:, u0 * 128:(u0 + nu) * 128],
                                  st[:, :nu * 128])

    nc.compile()
    return nc


def _build_launch2(prep):
    """Aggregate conv1 + fused D2 -> g2s shard rows."""
    nc = bacc.Bacc("TRN2", target_bir_lowering=False, debug=False,
                   num_devices=NCORES)
    GT = prep["GT"]
    Gc = prep["Gc"]

    tab1 = nc.dram_tensor("tab1", [TABR, F], F16, kind="ExternalInput")
    own1 = nc.dram_tensor("own1", [128, WPC * F], FP8, kind="ExternalInput")
    idxs = [nc.dram_tensor("idx%d" % ch, [128, int(Gc[ch]) * 8], I16,
                           kind="ExternalInput") for ch in range(NCHUNK)]
    dloc = nc.dram_tensor("dloc", [128, GT], F32, kind="ExternalInput")
    norm = nc.dram_tensor("norm", [128, GT], F32, kind="ExternalInput")
    dinv2 = nc.dram_tensor("dinv2", [128, WPC], F32, kind="ExternalInput")
    iota = nc.dram_tensor("iota", [128, 128], F16, kind="ExternalInput")
    iotac = nc.dram_tensor("iotac", [128, 1], F32, kind="ExternalInput")
    wfc2 = nc.dram_tensor("wfc2", [128, 128], F16, kind="ExternalInput")
    wc2 = nc.dram_tensor("wc2", [128, 128], F16, kind="ExternalInput")
    bc1 = nc.dram_tensor("bc1", [128, 1], F32, kind="ExternalInput")
    bfc2 = nc.dram_tensor("bfc2", [128, 1], F32, kind="ExternalInput")
    g2s = nc.dram_tensor("g2s", [SH, F], FP8, kind="ExternalOutput")

    with tile.TileContext(nc) as tc, contextlib.ExitStack() as ctx:
        idx_sb = _preload_idx(nc, tc, ctx, [a.ap() for a in idxs], prep)

        const = ctx.enter_context(tc.tile_pool(name="const", bufs=1))
        big = ctx.enter_context(tc.tile_pool(name="big", bufs=1))
        _n = [0]

        def ld(ap, shape, dtype):
            _n[0] += 1
            t = const.tile(shape, dtype, tag="c%d" % _n[0])
            nc.sync.dma_start(t[:], ap)
            return t

        dloc_sb = ld(dloc.ap(), [128, GT], F32)
        norm_sb = ld(norm.ap(), [128, GT], F32)
        own_sb = big.tile([128, WPC * F], FP8, tag="own")
        nc.scalar.dma_start(own_sb[:], own1.ap())
        dinv2_sb = ld(dinv2.ap(), [128, WPC], F32)
        iota_sb = ld(iota.ap(), [128, 128], F16)
        iotac_sb = ld(iotac.ap(), [128, 1], F32)
        wfc2_sb = ld(wfc2.ap(), [128, 128], F16)
        wc2_sb = ld(wc2.ap(), [128, 128], F16)
        bc1_sb = ld(bc1.ap(), [128, 1], F32)
        bfc2_sb = ld(bfc2.ap(), [128, 1], F32)
        h2T = big.tile([128, SH], F16, tag="h2T")

        winps = ctx.enter_context(tc.tile_pool(name="winps", bufs=2,
                                               space="PSUM"))
        mm1ps = ctx.enter_context(tc.tile_pool(name="mm1ps", bufs=2,
                                               space="PSUM"))
        tabps = ctx.enter_context(tc.tile_pool(name="tabps", bufs=2,
                                               space="PSUM"))
        hpool = ctx.enter_context(tc.tile_pool(name="hpool", bufs=3))
        stpool = ctx.enter_context(tc.tile_pool(name="stpool", bufs=3))

        g2s_pm = g2s.ap().rearrange("(s u) f -> s (u f)", s=128)
        d2state = {}

        def d2_tile(w):
            if w % 4 != 3 and w != WPC - 1:
                return
            u512 = w // 4
            c0 = u512 * 512
            cw = min(512, SH - c0)
            p1 = mm1ps.tile([128, 512], F32, tag="p1")
            nc.tensor.matmul(p1[:, :cw], wfc2_sb[:], h2T[:, c0:c0 + cw],
                             start=True, stop=True)
            h1 = hpool.tile([128, 512], F16, tag="h1")
            nc.scalar.activation(h1[:, :cw], p1[:, :cw], AF.Relu,
                                 bias=bfc2_sb[:, 0:1])
            tp2 = tabps.tile([128, 512], F32, tag="tp")
            for q in range(cw // 128):
                nc.tensor.matmul(
                    tp2[:, q * 128:(q + 1) * 128],
                    h1[:, q * 128:(q + 1) * 128], wc2_sb[:],
                    start=True, stop=True,
                )
            nu = cw // 128
            st = stpool.tile([128, 512], FP8, tag="st")
            nc.vector.tensor_copy(st[:, :nu * 128], tp2[:, :nu * 128])
            u0 = u512 * 4
            nc.sync.dma_start(g2s_pm[:, u0 * 128:(u0 + nu) * 128],
                              st[:, :nu * 128])

        _emit_agg(nc, tc, ctx, tab1.ap(), dloc_sb, norm_sb, iota_sb,
                  iotac_sb, dinv2_sb, own_sb, bc1_sb, h2T, prep, winps,
                  idx_sb, on_window=d2_tile)

    nc.compile()
    return nc


def _build_launch3(prep):
    """Aggregate conv2 + D3 + graph-pool partials."""
    nc = bacc.Bacc("TRN2", target_bir_lowering=False, debug=False,
                   num_devices=NCORES)
    GT = prep["GT"]
    Gc = prep["Gc"]

    tab2 = nc.dram_tensor("tab2", [TABR, F], F16, kind="ExternalInput")
    own2 = nc.dram_tensor("own2", [128, WPC * F], FP8, kind="ExternalInput")
    idxs = [nc.dram_tensor("idx%d" % ch, [128, int(Gc[ch]) * 8], I16,
                           kind="ExternalInput") for ch in range(NCHUNK)]
    dloc = nc.dram_tensor("dloc", [128, GT], F32, kind="ExternalInput")
    norm = nc.dram_tensor("norm", [128, GT], F32, kind="ExternalInput")
    dinv2 = nc.dram_tensor("dinv2", [128, WPC], F32, kind="ExternalInput")
    iota = nc.dram_tensor("iota", [128, 128], F16, kind="ExternalInput")
    iotac = nc.dram_tensor("iotac", [128, 1], F32, kind="ExternalInput")
    wfc = nc.dram_tensor("wfc", [128, NOUT], F16, kind="ExternalInput")
    bc2 = nc.dram_tensor("bc2", [128, 1], F32, kind="ExternalInput")
    g2d = nc.dram_tensor("g2d", [128, WPC * NG], F16, kind="ExternalInput")
    pool = nc.dram_tensor("pool", [NG, NOUT], F32, kind="ExternalOutput")

    with tile.TileContext(nc) as tc, contextlib.ExitStack() as ctx:
        idx_sb = _preload_idx(nc, tc, ctx, [a.ap() for a in idxs], prep)

        const = ctx.enter_context(tc.tile_pool(name="const", bufs=1))
        big = ctx.enter_context(tc.tile_pool(name="big", bufs=1))
        _n = [0]

        def ld(ap, shape, dtype):
            _n[0] += 1
            t = const.tile(shape, dtype, tag="c%d" % _n[0])
            nc.sync.dma_start(t[:], ap)
            return t

        dloc_sb = ld(dloc.ap(), [128, GT], F32)
        norm_sb = ld(norm.ap(), [128, GT], F32)
        own_sb = big.tile([128, WPC * F], FP8, tag="own")
        nc.scalar.dma_start(own_sb[:], own2.ap())
        dinv2_sb = ld(dinv2.ap(), [128, WPC], F32)
        iota_sb = ld(iota.ap(), [128, 128], F16)
        iotac_sb = ld(iotac.ap(), [128, 1], F32)
        wfc_sb = ld(wfc.ap(), [128, NOUT], F16)
        bc2_sb = ld(bc2.ap(), [128, 1], F32)
        g2d_sb = ld(g2d.ap(), [128, WPC * NG], F16)
        h4T = big.tile([128, SH], F16, tag="h4T")

        winps = ctx.enter_context(tc.tile_pool(name="winps", bufs=2,
                                               space="PSUM"))
        psd = ctx.enter_context(tc.tile_pool(name="psd", bufs=3, space="PSUM"))
        osb = ctx.enter_context(tc.tile_pool(name="osb", bufs=4))
        psp = ctx.enter_context(tc.tile_pool(name="psp", bufs=1, space="PSUM"))
        poolps = psp.tile([NG, NOUT], F32)

        def d3_win(w):
            pd = psd.tile([128, NOUT], F32, tag="pd")
            nc.tensor.matmul(pd[:], h4T[:, w * F:(w + 1) * F], wfc_sb[:],
                             start=True, stop=True)
            ot = osb.tile([128, NOUT], F16, tag="ot")
            nc.scalar.activation(ot[:], pd[:], AF.Copy)
            nc.tensor.matmul(poolps[:], g2d_sb[:, w * NG:(w + 1) * NG],
                             ot[:], start=(w == 0), stop=(w == WPC - 1),
                             skip_group_check=True)

        _emit_agg(nc, tc, ctx, tab2.ap(), dloc_sb, norm_sb, iota_sb,
                  iotac_sb, dinv2_sb, own_sb, bc2_sb, h4T, prep, winps,
                  idx_sb, on_window=d3_win)

        pres = osb.tile([NG, NOUT], F32, tag="pres")
        nc.vector.tensor_copy(pres[:], poolps[:])
        nc.sync.dma_start(pool.ap(), pres[:])

    nc.compile()
    return nc


def _np16(x):
    return np.ascontiguousarray(x, np.float16)


def _tl_ns(nc):
    from concourse.timeline_sim import TimelineSim
    tl = TimelineSim(nc, trace=False)
    tl.simulate()
    return int(tl.time)


def kernel(x, src, dst, batch, W_fc1, b_fc1, W_c1, b_c1, W_fc2, b_fc2, W_c2,
           b_c2, W_fc, b_fc):
    global LAST_EXEC_NS, LAST_INFO
    x = np.asarray(x, np.float32)
    prep = _prep(src, dst, batch)
    trace = os.environ.get("KERNEL_TRACE", "0") == "1"
    timing = os.environ.get("KERNEL_TIME", "0") == "1"

    col = lambda b: np.ascontiguousarray(
        np.asarray(b, np.float32).reshape(-1, 1))
    iota = np.tile(np.arange(128, dtype=np.float16), (128, 1))
    iotac = np.arange(128, dtype=np.float32).reshape(128, 1)

    # ---- launch 1: sharded dense -> g1s shards --------------------------
    nc1 = _build_launch1()
    in_maps1 = []
    for c in range(NCORES):
        import ml_dtypes
        pm = prep["perm"][c]
        xTc = np.zeros((SH, F), ml_dtypes.float8_e4m3fn)
        m = pm >= 0
        xTc[m] = x[pm[m]].astype(ml_dtypes.float8_e4m3fn)
        in_maps1.append({
            "xT": np.ascontiguousarray(xTc.T), "w1": _np16(W_fc1),
            "wc1": _np16(W_c1), "b1": col(b_fc1),
        })
    r1 = run_bass_kernel_spmd(nc1, in_maps1, core_ids=list(range(NCORES)),
                              trace=trace)
    t1 = r1.exec_time_ns or (_tl_ns(nc1) if timing else None)

    # host: assemble table1 + own views
    g1 = [np.asarray(r1.results[c]["g1s"]) for c in range(NCORES)]
    tab1 = np.zeros((TABR, F), np.float16)
    t1v = tab1.reshape(NCHUNK, NCORES, CSH, F)
    for c in range(NCORES):
        t1v[:, c] = g1[c].reshape(NCHUNK, CSH, F).astype(np.float16)
    tab1[prep["unocc_rows"]] = 0.0

    def own_view(gs):
        return np.ascontiguousarray(gs.reshape(128, WPC * F))

    def im_agg(c, tabname, tabv, ownv, extra):
        im = {
            tabname: tabv, "dloc": prep["dloc2d"][c], "norm": prep["norm2d"][c],
            "dinv2": np.ascontiguousarray(prep["dinv2"][c]),
            "iota": iota, "iotac": iotac,
        }
        for ch in range(NCHUNK):
            gc = int(prep["Gc"][ch]) * 8
            im["idx%d" % ch] = np.ascontiguousarray(
                prep["idx2d"][c, ch][:, :gc])
        im.update(extra)
        im["own1" if tabname == "tab1" else "own2"] = ownv
        return im

    # ---- launch 2: agg conv1 + D2 -> g2s shards -------------------------
    nc2 = _build_launch2(prep)
    in_maps2 = [
        im_agg(c, "tab1", tab1, own_view(g1[c]), {
            "wfc2": _np16(W_fc2), "wc2": _np16(W_c2),
            "bc1": col(b_c1), "bfc2": col(b_fc2),
        }) for c in range(NCORES)
    ]
    r2 = run_bass_kernel_spmd(nc2, in_maps2, core_ids=list(range(NCORES)),
                              trace=trace)
    t2 = r2.exec_time_ns or (_tl_ns(nc2) if timing else None)

    g2 = [np.asarray(r2.results[c]["g2s"]) for c in range(NCORES)]
    tab2 = np.zeros((TABR, F), np.float16)
    t2v = tab2.reshape(NCHUNK, NCORES, CSH, F)
    for c in range(NCORES):
        t2v[:, c] = g2[c].reshape(NCHUNK, CSH, F).astype(np.float16)
    tab2[prep["unocc_rows"]] = 0.0

    # ---- launch 3: agg conv2 + D3 + pool --------------------------------
    nc3 = _build_launch3(prep)
    in_maps3 = [
        im_agg(c, "tab2", tab2, own_view(g2[c]), {
            "wfc": _np16(W_fc), "bc2": col(b_c2),
            "g2d": np.ascontiguousarray(prep["g2d"][c]),
        }) for c in range(NCORES)
    ]
    r3 = run_bass_kernel_spmd(nc3, in_maps3, core_ids=list(range(NCORES)),
                              trace=trace)
    t3 = r3.exec_time_ns or (_tl_ns(nc3) if timing else None)

    out = np.zeros((NG, NOUT), np.float64)
    for c in range(NCORES):
        out += np.asarray(r3.results[c]["pool"]).astype(np.float64)
    out = out + np.asarray(b_fc, np.float64)[None, :]

    LAST_EXEC_NS = (t1 or 0) + (t2 or 0) + (t3 or 0)
    LAST_INFO = {"t1": t1, "t2": t2, "t3": t3, "GT": prep["GT"]}
    return out.astype(np.float32)


# revision 7
# speedup vs baseline: 1.2808x; 1.0096x over previous
"""2-layer GCN + dense layers + mean-pool on 8 trn2 NeuronCores (Bass/Tile).

v3 design (3 launches, sharded dense, self-loops via local diag matmuls).

GCNConv out = D^-1/2 (A+I) D^-1/2 (h W) + b factorizes as
  table[v]  = (h W)[v]                      (unscaled, per node)
  agg[d]    = sum_{e: dst=d} norm_e * table[src_e] + dinv_d^2 * table[d]
  h2[d]     = relu(agg[d] + b)
Self-loop terms never enter the gather stream: each core keeps its own dst
shard's table rows (slot-major, contiguous) in SBUF and adds them with one
matmul per window against a DVE-built diagonal (values dinv_d^2).

Node placement: greedy binning assigns each node to a (core, window) bin
(128 slots each) with per-chunk caps, where chunk class = node_id % 4 and
slots are class quarters (slot = class*32 + sloc).  Table row of node v:
  row(v) = class*25088 + core*3136 + sloc*98 + w   (= core shard row s*98+w)
so each core's dense output shard [12544, 128] is contiguous, gather chunks
(int16 idx) are fixed row ranges, and host-side assembly is pure reshape.

Launch 1: per-core dense D1 (x shard -> relu(xW1+b1) Wc1) -> shard rows.
Host: assemble table1, build own-shard views.  Launch 2: aggregate conv1
(dma_gather per 128-edge group + one-hot matmul, feature-major windows),
fused D2 -> table2 shard rows.  Host: assemble table2.  Launch 3: aggregate
conv2 + D3 + graph-pool partials; host sums partials + b_fc.

Both conv layers share one gather schedule (same graph, same row map).
"""

import os
import sys

sys.path.insert(0, "/opt/trn_rl_repo")

import contextlib

import numpy as np

import concourse.bass as bass
import concourse.tile as tile
from concourse import bacc, mybir
from concourse.bass_utils import run_bass_kernel_spmd

F32 = mybir.dt.float32
F16 = mybir.dt.float16
I16 = mybir.dt.int16
FP8 = mybir.dt.float8e4
AF = mybir.ActivationFunctionType
ALU = mybir.AluOpType

N = 100000
F = 128
NOUT = 64
NG = 64
NCORES = 8
WPC = 98                    # windows per core
WIN = 128
SH = WPC * WIN              # 12544 shard rows per core
NBINS = NCORES * WPC
NCHUNK = 4
CSH = SH // NCHUNK          # 3136 rows per (core, class)
CRE = NCORES * CSH          # 25088 rows per gather chunk
TABR = NCHUNK * CRE         # 100352 table rows
GCALL = 32                  # gather groups per dma_gather call

LAST_EXEC_NS = None
LAST_INFO = {}


# ----------------------------------------------------------------------------
# host-side graph prep
# ----------------------------------------------------------------------------
def _prep(src, dst, batch):
    src = np.asarray(src, np.int64)
    dst = np.asarray(dst, np.int64)
    batch = np.asarray(batch, np.int64)

    deg = np.bincount(dst, minlength=N).astype(np.float64) + 1.0
    dinv = 1.0 / np.sqrt(deg)

    cls = np.arange(N, dtype=np.int64) % NCHUNK
    k4 = np.zeros((N, NCHUNK), np.int64)
    np.add.at(k4, (dst, cls[src]), 1)
    ktot = k4.sum(1)

    # --- greedy binning: (core, window) bins, class quotas of 32 ----------
    def _greedy(margin, zadd):
        capG = {4: 4 * 128 - margin, 5: 5 * 128 - margin, 6: 6 * 128 - margin}
        share = k4.sum(0).max() / max(ktot.sum(), 1)
        Ecore = ktot.sum() / NCORES * 1.004
        Ty = capG[4] / share
        Tz = capG[5] / share
        z = int(np.ceil(max(0.0, (Ecore - WPC * Ty) / (Tz - Ty)))) + zadd
        z = min(max(z, 0), WPC)
        wclass = np.array([5] * z + [4] * (WPC - z))
        caps = np.zeros((NBINS, NCHUNK), np.float64)
        for b in range(NBINS):
            caps[b, :] = capG[wclass[b % WPC]]
        rem = caps.copy()
        mu = caps / 128.0
        slots = np.full(NBINS, 128, np.float64)
        clsroom = np.full((NBINS, NCHUNK), 32, np.int64)
        rng = np.random.default_rng(0)
        order = rng.permutation(N)
        bin_of = np.full(N, -1, np.int64)
        for v in order:
            need = k4[v]
            cl = cls[v]
            ok = (rem >= need).all(1) & (slots > 0) & (clsroom[:, cl] > 0)
            if not ok.any():
                ok = (slots > 0) & (clsroom[:, cl] > 0)
                if not ok.any():
                    ok = clsroom[:, cl] > 0
            dev = rem - need - (slots[:, None] - 1) * mu
            sc = np.where(ok, (dev * dev).sum(1), np.inf)
            b = int(np.argmin(sc))
            bin_of[v] = b
            rem[b] -= need
            slots[b] -= 1
            clsroom[b, cl] -= 1
        cntb = np.zeros((NBINS, NCHUNK), np.int64)
        for ch in range(NCHUNK):
            np.add.at(cntb[:, ch], bin_of, k4[:, ch])
        Gb = np.ceil(cntb.reshape(NCORES, WPC, NCHUNK).max(0) / 128.0)
        return bin_of, int(Gb.sum())

    best = None
    for margin, zadd in ((2, -1), (4, -1), (12, 0)):
        bo, gt = _greedy(margin, zadd)
        if best is None or gt < best[1]:
            best = (bo, gt)
        if gt <= 1576:
            break
    bin_of = best[0]

    core_of = bin_of // WPC
    w_of = bin_of % WPC

    # slots: class quarters; sloc = running fill per (bin, class)
    sloc_of = np.empty(N, np.int64)
    fill = np.zeros((NBINS, NCHUNK), np.int64)
    sidx = np.argsort(bin_of, kind="stable")
    for v in sidx:
        b, cl = bin_of[v], cls[v]
        sloc_of[v] = fill[b, cl]
        fill[b, cl] += 1
    assert fill.max() <= 32
    s_of = cls * 32 + sloc_of                     # global slot 0..127
    row = cls * CRE + core_of * CSH + sloc_of * 98 + w_of   # table row
    shrow = s_of * WPC + w_of                     # shard-local row

    # unoccupied (c, w, s) slots -> zero rows / pad gather targets
    occ = np.zeros((NCORES, WPC, WIN), bool)
    occ[core_of, w_of, s_of] = True
    uc, uw, us = np.nonzero(~occ)
    unocc_rows = ((us // 32) * CRE + uc * CSH + (us % 32) * 98 + uw)
    pad_iloc = np.zeros(NCHUNK, np.int64)
    for ch in range(NCHUNK):
        cand = unocc_rows[(unocc_rows >= ch * CRE) & (unocc_rows < (ch + 1) * CRE)]
        assert len(cand) > 0, ch
        pad_iloc[ch] = cand[0] % CRE

    # --- per-core edge streams (no self-loops in stream) ------------------
    e_core = core_of[dst]
    e_w = w_of[dst]
    e_slot = s_of[dst].astype(np.float32)
    e_ch = cls[src]
    iloc = (row[src] % CRE).astype(np.int16)
    norm = (dinv[src] * dinv[dst]).astype(np.float32)

    key = (e_core * NCHUNK + e_ch) * WPC + e_w
    nk = NCORES * NCHUNK * WPC
    cnt = np.bincount(key, minlength=nk).reshape(NCORES, NCHUNK, WPC)
    G = np.ceil(cnt.max(axis=0) / 128.0).astype(np.int64)   # [NCHUNK, WPC]
    Gc = G.sum(axis=1)
    GT = int(G.sum())

    # emission order (w, ch, j); chunk-local gather order is (w, j)
    gbase = np.zeros((WPC, NCHUNK), np.int64)
    run = 0
    for w in range(WPC):
        for ch in range(NCHUNK):
            gbase[w, ch] = run
            run += G[ch, w]
    assert run == GT
    cbase = np.zeros((WPC, NCHUNK), np.int64)
    crun = np.zeros(NCHUNK, np.int64)
    for w in range(WPC):
        for ch in range(NCHUNK):
            cbase[w, ch] = crun[ch]
            crun[ch] += G[ch, w]
    assert (crun == Gc).all()

    eorder = np.lexsort((e_ch, e_w, e_core))
    key_s = ((e_core * WPC + e_w) * NCHUNK + e_ch)[eorder]
    iloc_s = iloc[eorder]
    slot_s = e_slot[eorder]
    norm_s = norm[eorder]
    bounds = np.searchsorted(key_s, np.arange(NCORES * WPC * NCHUNK + 1))

    mg = max(int(c) for c in Gc) * 128
    idx_streams = np.zeros((NCORES, NCHUNK, mg), np.int16)
    for ch in range(NCHUNK):
        idx_streams[:, ch, :] = pad_iloc[ch]
    dloc2d = np.full((NCORES, 128, GT), -1.0, np.float32)
    norm2d = np.zeros((NCORES, 128, GT), np.float32)
    for c in range(NCORES):
        for w in range(WPC):
            for ch in range(NCHUNK):
                k = (c * WPC + w) * NCHUNK + ch
                b0, b1 = bounds[k], bounds[k + 1]
                n = b1 - b0
                g = int(G[ch, w])
                assert n <= g * 128
                co = int(cbase[w, ch]) * 128
                idx_streams[c, ch, co:co + n] = iloc_s[b0:b1]
                gg = int(gbase[w, ch])
                sl = np.full(g * 128, -1.0, np.float32)
                nv = np.zeros(g * 128, np.float32)
                sl[:n] = slot_s[b0:b1]
                nv[:n] = norm_s[b0:b1]
                dloc2d[c, :, gg:gg + g] = sl.reshape(g, 128).T
                norm2d[c, :, gg:gg + g] = nv.reshape(g, 128).T

    idx2d = np.zeros((NCORES, NCHUNK, 128, mg // 16), np.int16)
    for c in range(NCORES):
        for ch in range(NCHUNK):
            a = idx_streams[c, ch].reshape(-1, 16).T
            idx2d[c, ch] = np.tile(a, (8, 1))

    # --- per-core aux tensors --------------------------------------------
    nodes = np.arange(N)
    perm = np.full((NCORES, SH), -1, np.int64)    # col i = w*128+s -> node
    perm[core_of, w_of * WIN + s_of] = nodes

    dinv2 = np.zeros((NCORES, WIN, WPC), np.float32)
    dinv2[core_of, s_of, w_of] = (dinv[nodes] ** 2).astype(np.float32)

    counts = np.maximum(np.bincount(batch, minlength=NG), 1).astype(np.float64)
    g2d = np.zeros((NCORES, WIN, WPC * NG), np.float16)
    g2d[core_of, s_of, w_of * NG + batch[nodes]] = (
        1.0 / counts[batch[nodes]]).astype(np.float16)

    return dict(
        G=G, Gc=Gc, GT=GT, gbase=gbase, cbase=cbase,
        idx2d=idx2d, dloc2d=dloc2d, norm2d=norm2d,
        core_of=core_of, w_of=w_of, s_of=s_of, row=row, shrow=shrow,
        perm=perm, dinv2=dinv2, g2d=g2d, counts=counts,
        unocc_rows=np.sort(unocc_rows), pad_iloc=pad_iloc,
    )


# ----------------------------------------------------------------------------
# device program pieces
# ----------------------------------------------------------------------------
def _preload_idx(nc, tc, ctx, idx_aps, sched):
    Gc = sched["Gc"]
    idxc = ctx.enter_context(tc.tile_pool(name="idxc", bufs=1))
    idx_sb = {}
    for ch in range(NCHUNK):
        if Gc[ch] == 0:
            continue
        it = idxc.tile([128, int(Gc[ch]) * 8], I16, tag=f"idx{ch}")
        nc.sync.dma_start(it[:], idx_aps[ch])
        idx_sb[ch] = it
    return idx_sb


def _emit_agg(nc, tc, ctx, tab_ap, dloc_sb, norm_sb, iota_sb, iotacol_sb,
              dinv2_sb, own_sb, bias_sb, hT, sched, winps, idx_sb,
              on_window=None):
    """Aggregate edges + per-window self-loop diag matmul.
    hT[:, w*128:(w+1)*128] = relu(agg_w + bias), feature-major."""
    G, Gc = sched["G"], sched["Gc"]

    gath = {}
    for ch in range(NCHUNK):
        if Gc[ch] == 0:
            continue
        gath[ch] = ctx.enter_context(tc.tile_pool(name=f"gath{ch}", bufs=3))

    tiles = {ch: [] for ch in range(NCHUNK)}
    issued = {ch: 0 for ch in range(NCHUNK)}

    def ensure(ch, upto):
        while issued[ch] <= upto:
            g0 = issued[ch]
            rem = Gc[ch] - g0
            ng = int(min(GCALL if rem > 2 * GCALL else GCALL // 2, rem))
            gt = gath[ch].tile([128, GCALL * F], F16, tag="gt")
            base = ch * CRE
            nc.gpsimd.dma_gather(
                gt[:, :ng * F].rearrange("p (g e) -> p g e", e=F),
                tab_ap[base:base + CRE, :],
                idx_sb[ch][:, g0 * 8:(g0 + ng) * 8], ng * 128, ng * 128, F,
                single_packet=False,
            )
            tiles[ch].append((gt, g0, ng))
            issued[ch] += ng

    ohp = ctx.enter_context(tc.tile_pool(name="ohp", bufs=20))
    gbase, cbase = sched["gbase"], sched["cbase"]
    for w in range(WPC):
        wt = winps.tile([128, 128], F32, tag="win")
        # self-loop diag: oh[s, j] = (j == s) * dinv2[s, w]
        ohs = ohp.tile([128, 128], F16, tag="oh")
        nc.vector.tensor_scalar(
            ohs[:], iota_sb[:], iotacol_sb[:, 0:1], dinv2_sb[:, w:w + 1],
            ALU.is_equal, ALU.mult,
        )
        nc.tensor.matmul(wt[:], own_sb[:, w * F:(w + 1) * F], ohs[:],
                         start=True, stop=False)
        total = int(sum(G[ch, w] for ch in range(NCHUNK)))
        done = 0
        for ch in range(NCHUNK):
            g = int(G[ch, w])
            for j in range(g):
                cg = int(cbase[w, ch]) + j
                ensure(ch, cg)
                gt, g0, ng = next(
                    t for t in tiles[ch] if t[1] <= cg < t[1] + t[2])
                k = cg - g0
                gg = int(gbase[w, ch]) + j
                oh = ohp.tile([128, 128], F16, tag="oh")
                nc.vector.tensor_scalar(
                    oh[:], iota_sb[:], dloc_sb[:, gg:gg + 1],
                    norm_sb[:, gg:gg + 1], ALU.is_equal, ALU.mult,
                )
                done += 1
                nc.tensor.matmul(
                    wt[:], gt[:, k * F:(k + 1) * F], oh[:],
                    start=False, stop=(done == total),
                )
        nc.scalar.activation(hT[:, w * F:(w + 1) * F], wt[:], AF.Relu,
                             bias=bias_sb[:, 0:1])
        if on_window is not None:
            on_window(w)


# ----------------------------------------------------------------------------
# builders
# ----------------------------------------------------------------------------
def _build_launch1():
    """Sharded dense D1: xTc [128, SH] -> g1s [SH, 128] (rows s*98+w)."""
    nc = bacc.Bacc("TRN2", target_bir_lowering=False, debug=False,
                   num_devices=NCORES)
    xT = nc.dram_tensor("xT", [128, SH], FP8, kind="ExternalInput")
    w1 = nc.dram_tensor("w1", [128, 128], F16, kind="ExternalInput")
    wc1 = nc.dram_tensor("wc1", [128, 128], F16, kind="ExternalInput")
    b1 = nc.dram_tensor("b1", [128, 1], F32, kind="ExternalInput")
    g1s = nc.dram_tensor("g1s", [SH, F], FP8, kind="ExternalOutput")

    with tile.TileContext(nc) as tc, contextlib.ExitStack() as ctx:
        const = ctx.enter_context(tc.tile_pool(name="const", bufs=1))
        w1_sb = const.tile([128, 128], F16, tag="w1")
        nc.sync.dma_start(w1_sb[:], w1.ap())
        wc1_sb = const.tile([128, 128], F16, tag="wc1")
        nc.sync.dma_start(wc1_sb[:], wc1.ap())
        b1_sb = const.tile([128, 1], F32, tag="b1")
        nc.sync.dma_start(b1_sb[:], b1.ap())

        mm1ps = ctx.enter_context(tc.tile_pool(name="mm1ps", bufs=2,
                                               space="PSUM"))
        tabps = ctx.enter_context(tc.tile_pool(name="tabps", bufs=2,
                                               space="PSUM"))
        hpool = ctx.enter_context(tc.tile_pool(name="hpool", bufs=3))
        stpool = ctx.enter_context(tc.tile_pool(name="stpool", bufs=3))
        xin = ctx.enter_context(tc.tile_pool(name="xin", bufs=3))

        g1s_pm = g1s.ap().rearrange("(s u) f -> s (u f)", s=128)
        xt_cache = {}

        def src1(u512):
            blk = u512 // 4
            if blk not in xt_cache:
                t = xin.tile([128, 2048], FP8, tag="xt")
                cw = min(2048, SH - blk * 2048)
                nc.scalar.dma_start(t[:, :cw], xT.ap()[:, blk * 2048:blk * 2048 + cw])
                xt_cache.clear()
                xt_cache[blk] = t
            return xt_cache[blk]

        NT = (SH + 511) // 512          # 25 tiles, last = 256 cols
        st = None
        for u512 in range(NT):
            c0 = u512 * 512
            cw = min(512, SH - c0)
            xt = src1(u512)
            xs = xt[:, (u512 % 4) * 512:(u512 % 4) * 512 + cw]
            p1 = mm1ps.tile([128, 512], F32, tag="p1")
            nc.tensor.matmul(p1[:, :cw], w1_sb[:], xs, start=True, stop=True)
            h1 = hpool.tile([128, 512], F16, tag="h1")
            nc.scalar.activation(h1[:, :cw], p1[:, :cw], AF.Relu,
                                 bias=b1_sb[:, 0:1])
            q8 = u512 % 2
            if q8 == 0:
                st = stpool.tile([128, 1024], FP8, tag="st")
                tp = tabps.tile([128, 1024], F32, tag="tp")
                _build_launch1.tp = tp
            tp = _build_launch1.tp
            for q in range(cw // 128):
                nc.tensor.matmul(
                    tp[:, (q8 * 4 + q) * 128:(q8 * 4 + q + 1) * 128],
                    h1[:, q * 128:(q + 1) * 128], wc1_sb[:],
                    start=True, stop=True,
                )
            if q8 == 1 or u512 == NT - 1:
                nu = q8 * 4 + cw // 128
                nc.vector.tensor_copy(st[:, :nu * 128], tp[:, :nu * 128])
                u0 = (u512 // 2) * 8
                nc.sync.dma_start(g1s_pm[:, u0 * 128:(u0 + nu) * 128],
                                  st[:, :nu * 128])

    nc.compile()
    return nc


def _build_launch2(prep):
    """Aggregate conv1 + fused D2 -> g2s shard rows."""
    nc = bacc.Bacc("TRN2", target_bir_lowering=False, debug=False,
                   num_devices=NCORES)
    GT = prep["GT"]
    Gc = prep["Gc"]

    tab1 = nc.dram_tensor("tab1", [TABR, F], F16, kind="ExternalInput")
    own1 = nc.dram_tensor("own1", [128, WPC * F], FP8, kind="ExternalInput")
    idxs = [nc.dram_tensor("idx%d" % ch, [128, int(Gc[ch]) * 8], I16,
                           kind="ExternalInput") for ch in range(NCHUNK)]
    dloc = nc.dram_tensor("dloc", [128, GT], F32, kind="ExternalInput")
    norm = nc.dram_tensor("norm", [128, GT], F32, kind="ExternalInput")
    dinv2 = nc.dram_tensor("dinv2", [128, WPC], F32, kind="ExternalInput")
    iota = nc.dram_tensor("iota", [128, 128], F16, kind="ExternalInput")
    iotac = nc.dram_tensor("iotac", [128, 1], F32, kind="ExternalInput")
    wfc2 = nc.dram_tensor("wfc2", [128, 128], F16, kind="ExternalInput")
    wc2 = nc.dram_tensor("wc2", [128, 128], F16, kind="ExternalInput")
    bc1 = nc.dram_tensor("bc1", [128, 1], F32, kind="ExternalInput")
    bfc2 = nc.dram_tensor("bfc2", [128, 1], F32, kind="ExternalInput")
    g2s = nc.dram_tensor("g2s", [SH, F], FP8, kind="ExternalOutput")

    with tile.TileContext(nc) as tc, contextlib.ExitStack() as ctx:
        const = ctx.enter_context(tc.tile_pool(name="const", bufs=1))
        big = ctx.enter_context(tc.tile_pool(name="big", bufs=1))
        _n = [0]

        def ld(ap, shape, dtype):
            _n[0] += 1
            t = const.tile(shape, dtype, tag="c%d" % _n[0])
            nc.sync.dma_start(t[:], ap)
            return t

        dinv2_sb = ld(dinv2.ap(), [128, WPC], F32)
        iota_sb = ld(iota.ap(), [128, 128], F16)
        iotac_sb = ld(iotac.ap(), [128, 1], F32)
        wfc2_sb = ld(wfc2.ap(), [128, 128], F16)
        wc2_sb = ld(wc2.ap(), [128, 128], F16)
        bc1_sb = ld(bc1.ap(), [128, 1], F32)
        bfc2_sb = ld(bfc2.ap(), [128, 1], F32)
        dloc_sb = ld(dloc.ap(), [128, GT], F32)
        norm_sb = ld(norm.ap(), [128, GT], F32)
        own_sb = big.tile([128, WPC * F], FP8, tag="own")
        nc.sync.dma_start(own_sb[:], own1.ap())
        idx_sb = _preload_idx(nc, tc, ctx, [a.ap() for a in idxs], prep)
        h2T = big.tile([128, SH], F16, tag="h2T")

        winps = ctx.enter_context(tc.tile_pool(name="winps", bufs=2,
                                               space="PSUM"))
        mm1ps = ctx.enter_context(tc.tile_pool(name="mm1ps", bufs=2,
                                               space="PSUM"))
        tabps = ctx.enter_context(tc.tile_pool(name="tabps", bufs=2,
                                               space="PSUM"))
        hpool = ctx.enter_context(tc.tile_pool(name="hpool", bufs=3))
        stpool = ctx.enter_context(tc.tile_pool(name="stpool", bufs=3))

        g2s_pm = g2s.ap().rearrange("(s u) f -> s (u f)", s=128)
        d2state = {}

        def d2_tile(w):
            if w % 4 != 3 and w != WPC - 1:
                return
            u512 = w // 4
            c0 = u512 * 512
            cw = min(512, SH - c0)
            p1 = mm1ps.tile([128, 512], F32, tag="p1")
            nc.tensor.matmul(p1[:, :cw], wfc2_sb[:], h2T[:, c0:c0 + cw],
                             start=True, stop=True)
            h1 = hpool.tile([128, 512], F16, tag="h1")
            nc.scalar.activation(h1[:, :cw], p1[:, :cw], AF.Relu,
                                 bias=bfc2_sb[:, 0:1])
            tp2 = tabps.tile([128, 512], F32, tag="tp")
            for q in range(cw // 128):
                nc.tensor.matmul(
                    tp2[:, q * 128:(q + 1) * 128],
                    h1[:, q * 128:(q + 1) * 128], wc2_sb[:],
                    start=True, stop=True,
                )
            nu = cw // 128
            st = stpool.tile([128, 512], FP8, tag="st")
            nc.vector.tensor_copy(st[:, :nu * 128], tp2[:, :nu * 128])
            u0 = u512 * 4
            nc.sync.dma_start(g2s_pm[:, u0 * 128:(u0 + nu) * 128],
                              st[:, :nu * 128])

        _emit_agg(nc, tc, ctx, tab1.ap(), dloc_sb, norm_sb, iota_sb,
                  iotac_sb, dinv2_sb, own_sb, bc1_sb, h2T, prep, winps,
                  idx_sb, on_window=d2_tile)

    nc.compile()
    return nc


def _build_launch3(prep):
    """Aggregate conv2 + D3 + graph-pool partials."""
    nc = bacc.Bacc("TRN2", target_bir_lowering=False, debug=False,
                   num_devices=NCORES)
    GT = prep["GT"]
    Gc = prep["Gc"]

    tab2 = nc.dram_tensor("tab2", [TABR, F], F16, kind="ExternalInput")
    own2 = nc.dram_tensor("own2", [128, WPC * F], FP8, kind="ExternalInput")
    idxs = [nc.dram_tensor("idx%d" % ch, [128, int(Gc[ch]) * 8], I16,
                           kind="ExternalInput") for ch in range(NCHUNK)]
    dloc = nc.dram_tensor("dloc", [128, GT], F32, kind="ExternalInput")
    norm = nc.dram_tensor("norm", [128, GT], F32, kind="ExternalInput")
    dinv2 = nc.dram_tensor("dinv2", [128, WPC], F32, kind="ExternalInput")
    iota = nc.dram_tensor("iota", [128, 128], F16, kind="ExternalInput")
    iotac = nc.dram_tensor("iotac", [128, 1], F32, kind="ExternalInput")
    wfc = nc.dram_tensor("wfc", [128, NOUT], F16, kind="ExternalInput")
    bc2 = nc.dram_tensor("bc2", [128, 1], F32, kind="ExternalInput")
    g2d = nc.dram_tensor("g2d", [128, WPC * NG], F16, kind="ExternalInput")
    pool = nc.dram_tensor("pool", [NG, NOUT], F32, kind="ExternalOutput")

    with tile.TileContext(nc) as tc, contextlib.ExitStack() as ctx:
        const = ctx.enter_context(tc.tile_pool(name="const", bufs=1))
        big = ctx.enter_context(tc.tile_pool(name="big", bufs=1))
        _n = [0]

        def ld(ap, shape, dtype):
            _n[0] += 1
            t = const.tile(shape, dtype, tag="c%d" % _n[0])
            nc.sync.dma_start(t[:], ap)
            return t

        dinv2_sb = ld(dinv2.ap(), [128, WPC], F32)
        iota_sb = ld(iota.ap(), [128, 128], F16)
        iotac_sb = ld(iotac.ap(), [128, 1], F32)
        wfc_sb = ld(wfc.ap(), [128, NOUT], F16)
        bc2_sb = ld(bc2.ap(), [128, 1], F32)
        g2d_sb = ld(g2d.ap(), [128, WPC * NG], F16)
        dloc_sb = ld(dloc.ap(), [128, GT], F32)
        norm_sb = ld(norm.ap(), [128, GT], F32)
        own_sb = big.tile([128, WPC * F], FP8, tag="own")
        nc.sync.dma_start(own_sb[:], own2.ap())
        idx_sb = _preload_idx(nc, tc, ctx, [a.ap() for a in idxs], prep)
        h4T = big.tile([128, SH], F16, tag="h4T")

        winps = ctx.enter_context(tc.tile_pool(name="winps", bufs=2,
                                               space="PSUM"))
        psd = ctx.enter_context(tc.tile_pool(name="psd", bufs=3, space="PSUM"))
        osb = ctx.enter_context(tc.tile_pool(name="osb", bufs=4))
        psp = ctx.enter_context(tc.tile_pool(name="psp", bufs=1, space="PSUM"))
        poolps = psp.tile([NG, NOUT], F32)

        def d3_win(w):
            pd = psd.tile([128, NOUT], F32, tag="pd")
            nc.tensor.matmul(pd[:], h4T[:, w * F:(w + 1) * F], wfc_sb[:],
                             start=True, stop=True)
            ot = osb.tile([128, NOUT], F16, tag="ot")
            nc.scalar.activation(ot[:], pd[:], AF.Copy)
            nc.tensor.matmul(poolps[:], g2d_sb[:, w * NG:(w + 1) * NG],
                             ot[:], start=(w == 0), stop=(w == WPC - 1),
                             skip_group_check=True)

        _emit_agg(nc, tc, ctx, tab2.ap(), dloc_sb, norm_sb, iota_sb,
                  iotac_sb, dinv2_sb, own_sb, bc2_sb, h4T, prep, winps,
                  idx_sb, on_window=d3_win)

        pres = osb.tile([NG, NOUT], F32, tag="pres")
        nc.vector.tensor_copy(pres[:], poolps[:])
        nc.sync.dma_start(pool.ap(), pres[:])

    nc.compile()
    return nc


def _np16(x):
    return np.ascontiguousarray(x, np.float16)


def _tl_ns(nc):
    from concourse.timeline_sim import TimelineSim
    tl = TimelineSim(nc, trace=False)
    tl.simulate()
    return int(tl.time)


def kernel(x, src, dst, batch, W_fc1, b_fc1, W_c1, b_c1, W_fc2, b_fc2, W_c2,
           b_c2, W_fc, b_fc):
    global LAST_EXEC_NS, LAST_INFO
    x = np.asarray(x, np.float32)
    prep = _prep(src, dst, batch)
    trace = os.environ.get("KERNEL_TRACE", "0") == "1"
    timing = os.environ.get("KERNEL_TIME", "0") == "1"

    col = lambda b: np.ascontiguousarray(
        np.asarray(b, np.float32).reshape(-1, 1))
    iota = np.tile(np.arange(128, dtype=np.float16), (128, 1))
    iotac = np.arange(128, dtype=np.float32).reshape(128, 1)

    # ---- launch 1: sharded dense -> g1s shards --------------------------
    nc1 = _build_launch1()
    in_maps1 = []
    for c in range(NCORES):
        import ml_dtypes
        pm = prep["perm"][c]
        xTc = np.zeros((SH, F), ml_dtypes.float8_e4m3fn)
        m = pm >= 0
        xTc[m] = x[pm[m]].astype(ml_dtypes.float8_e4m3fn)
        in_maps1.append({
            "xT": np.ascontiguousarray(xTc.T), "w1": _np16(W_fc1),
            "wc1": _np16(W_c1), "b1": col(b_fc1),
        })
    r1 = run_bass_kernel_spmd(nc1, in_maps1, core_ids=list(range(NCORES)),
                              trace=trace)
    t1 = r1.exec_time_ns or (_tl_ns(nc1) if timing else None)

    # host: assemble table1 + own views
    g1 = [np.asarray(r1.results[c]["g1s"]) for c in range(NCORES)]
    tab1 = np.zeros((TABR, F), np.float16)
    t1v = tab1.reshape(NCHUNK, NCORES, CSH, F)
    for c in range(NCORES):
        t1v[:, c] = g1[c].reshape(NCHUNK, CSH, F).astype(np.float16)
    tab1[prep["unocc_rows"]] = 0.0

    def own_view(gs):
        return np.ascontiguousarray(gs.reshape(128, WPC * F))

    def im_agg(c, tabname, tabv, ownv, extra):
        im = {
            tabname: tabv, "dloc": prep["dloc2d"][c], "norm": prep["norm2d"][c],
            "dinv2": np.ascontiguousarray(prep["dinv2"][c]),
            "iota": iota, "iotac": iotac,
        }
        for ch in range(NCHUNK):
            gc = int(prep["Gc"][ch]) * 8
            im["idx%d" % ch] = np.ascontiguousarray(
                prep["idx2d"][c, ch][:, :gc])
        im.update(extra)
        im["own1" if tabname == "tab1" else "own2"] = ownv
        return im

    # ---- launch 2: agg conv1 + D2 -> g2s shards -------------------------
    nc2 = _build_launch2(prep)
    in_maps2 = [
        im_agg(c, "tab1", tab1, own_view(g1[c]), {
            "wfc2": _np16(W_fc2), "wc2": _np16(W_c2),
            "bc1": col(b_c1), "bfc2": col(b_fc2),
        }) for c in range(NCORES)
    ]
    r2 = run_bass_kernel_spmd(nc2, in_maps2, core_ids=list(range(NCORES)),
                              trace=trace)
    t2 = r2.exec_time_ns or (_tl_ns(nc2) if timing else None)

    g2 = [np.asarray(r2.results[c]["g2s"]) for c in range(NCORES)]
    tab2 = np.zeros((TABR, F), np.float16)
    t2v = tab2.reshape(NCHUNK, NCORES, CSH, F)
    for c in range(NCORES):
        t2v[:, c] = g2[c].reshape(NCHUNK, CSH, F).astype(np.float16)
    tab2[prep["unocc_rows"]] = 0.0

    # ---- launch 3: agg conv2 + D3 + pool --------------------------------
    nc3 = _build_launch3(prep)
    in_maps3 = [
        im_agg(c, "tab2", tab2, own_view(g2[c]), {
            "wfc": _np16(W_fc), "bc2": col(b_c2),
            "g2d": np.ascontiguousarray(prep["g2d"][c]),
        }) for c in range(NCORES)
    ]
    r3 = run_bass_kernel_spmd(nc3, in_maps3, core_ids=list(range(NCORES)),
                              trace=trace)
    t3 = r3.exec_time_ns or (_tl_ns(nc3) if timing else None)

    out = np.zeros((NG, NOUT), np.float64)
    for c in range(NCORES):
        out += np.asarray(r3.results[c]["pool"]).astype(np.float64)
    out = out + np.asarray(b_fc, np.float64)[None, :]

    LAST_EXEC_NS = (t1 or 0) + (t2 or 0) + (t3 or 0)
    LAST_INFO = {"t1": t1, "t2": t2, "t3": t3, "GT": prep["GT"]}
    return out.astype(np.float32)


# revision 10
# speedup vs baseline: 1.2919x; 1.0087x over previous
"""2-layer GCN + dense layers + mean-pool on 8 trn2 NeuronCores (Bass/Tile).

v3 design (3 launches, sharded dense, self-loops via local diag matmuls).

GCNConv out = D^-1/2 (A+I) D^-1/2 (h W) + b factorizes as
  table[v]  = (h W)[v]                      (unscaled, per node)
  agg[d]    = sum_{e: dst=d} norm_e * table[src_e] + dinv_d^2 * table[d]
  h2[d]     = relu(agg[d] + b)
Self-loop terms never enter the gather stream: each core keeps its own dst
shard's table rows (slot-major, contiguous) in SBUF and adds them with one
matmul per window against a DVE-built diagonal (values dinv_d^2).

Node placement: greedy binning assigns each node to a (core, window) bin
(128 slots each) with per-chunk caps, where chunk class = node_id % 4 and
slots are class quarters (slot = class*32 + sloc).  Table row of node v:
  row(v) = class*25088 + core*3136 + sloc*98 + w   (= core shard row s*98+w)
so each core's dense output shard [12544, 128] is contiguous, gather chunks
(int16 idx) are fixed row ranges, and host-side assembly is pure reshape.

Launch 1: per-core dense D1 (x shard -> relu(xW1+b1) Wc1) -> shard rows.
Host: assemble table1, build own-shard views.  Launch 2: aggregate conv1
(dma_gather per 128-edge group + one-hot matmul, feature-major windows),
fused D2 -> table2 shard rows.  Host: assemble table2.  Launch 3: aggregate
conv2 + D3 + graph-pool partials; host sums partials + b_fc.

Both conv layers share one gather schedule (same graph, same row map).
"""

import os
import sys

sys.path.insert(0, "/opt/trn_rl_repo")

import contextlib

import numpy as np

import concourse.bass as bass
import concourse.tile as tile
from concourse import bacc, mybir
from concourse.bass_utils import run_bass_kernel_spmd

F32 = mybir.dt.float32
F16 = mybir.dt.float16
I16 = mybir.dt.int16
FP8 = mybir.dt.float8e4
AF = mybir.ActivationFunctionType
ALU = mybir.AluOpType

N = 100000
F = 128
NOUT = 64
NG = 64
NCORES = 8
WPC = 98                    # windows per core
WIN = 128
SH = WPC * WIN              # 12544 shard rows per core
NBINS = NCORES * WPC
NCHUNK = 4
CSH = SH // NCHUNK          # 3136 rows per (core, class)
CRE = NCORES * CSH          # 25088 rows per gather chunk
TABR = NCHUNK * CRE         # 100352 table rows
GCALL = 32                  # gather groups per dma_gather call

LAST_EXEC_NS = None
LAST_INFO = {}


# ----------------------------------------------------------------------------
# host-side graph prep
# ----------------------------------------------------------------------------
def _prep(src, dst, batch):
    src = np.asarray(src, np.int64)
    dst = np.asarray(dst, np.int64)
    batch = np.asarray(batch, np.int64)

    deg = np.bincount(dst, minlength=N).astype(np.float64) + 1.0
    dinv = 1.0 / np.sqrt(deg)

    cls = np.arange(N, dtype=np.int64) % NCHUNK
    k4 = np.zeros((N, NCHUNK), np.int64)
    np.add.at(k4, (dst, cls[src]), 1)
    ktot = k4.sum(1)

    # --- greedy binning: (core, window) bins, class quotas of 32 ----------
    def _greedy(margin, zadd):
        capG = {4: 4 * 128 - margin, 5: 5 * 128 - margin, 6: 6 * 128 - margin}
        share = k4.sum(0).max() / max(ktot.sum(), 1)
        Ecore = ktot.sum() / NCORES * 1.004
        Ty = capG[4] / share
        Tz = capG[5] / share
        z = int(np.ceil(max(0.0, (Ecore - WPC * Ty) / (Tz - Ty)))) + zadd
        z = min(max(z, 0), WPC)
        wclass = np.array([5] * z + [4] * (WPC - z))
        caps = np.zeros((NBINS, NCHUNK), np.float64)
        for b in range(NBINS):
            caps[b, :] = capG[wclass[b % WPC]]
        rem = caps.copy()
        mu = caps / 128.0
        slots = np.full(NBINS, 128, np.float64)
        clsroom = np.full((NBINS, NCHUNK), 32, np.int64)
        rng = np.random.default_rng(0)
        order = rng.permutation(N)
        bin_of = np.full(N, -1, np.int64)
        for v in order:
            need = k4[v]
            cl = cls[v]
            ok = (rem >= need).all(1) & (slots > 0) & (clsroom[:, cl] > 0)
            if not ok.any():
                ok = (slots > 0) & (clsroom[:, cl] > 0)
                if not ok.any():
                    ok = clsroom[:, cl] > 0
            dev = rem - need - (slots[:, None] - 1) * mu
            sc = np.where(ok, (dev * dev).sum(1), np.inf)
            b = int(np.argmin(sc))
            bin_of[v] = b
            rem[b] -= need
            slots[b] -= 1
            clsroom[b, cl] -= 1
        cntb = np.zeros((NBINS, NCHUNK), np.int64)
        for ch in range(NCHUNK):
            np.add.at(cntb[:, ch], bin_of, k4[:, ch])
        Gb = np.ceil(cntb.reshape(NCORES, WPC, NCHUNK).max(0) / 128.0)
        return bin_of, int(Gb.sum())

    best = None
    for margin, zadd in ((2, -1), (4, -1), (12, 0)):
        bo, gt = _greedy(margin, zadd)
        if best is None or gt < best[1]:
            best = (bo, gt)
        if gt <= 1576:
            break
    bin_of = best[0]

    core_of = bin_of // WPC
    w_of = bin_of % WPC

    # slots: class quarters; sloc = running fill per (bin, class)
    sloc_of = np.empty(N, np.int64)
    fill = np.zeros((NBINS, NCHUNK), np.int64)
    sidx = np.argsort(bin_of, kind="stable")
    for v in sidx:
        b, cl = bin_of[v], cls[v]
        sloc_of[v] = fill[b, cl]
        fill[b, cl] += 1
    assert fill.max() <= 32
    s_of = cls * 32 + sloc_of                     # global slot 0..127
    row = cls * CRE + core_of * CSH + sloc_of * 98 + w_of   # table row
    shrow = s_of * WPC + w_of                     # shard-local row

    # unoccupied (c, w, s) slots -> zero rows / pad gather targets
    occ = np.zeros((NCORES, WPC, WIN), bool)
    occ[core_of, w_of, s_of] = True
    uc, uw, us = np.nonzero(~occ)
    unocc_rows = ((us // 32) * CRE + uc * CSH + (us % 32) * 98 + uw)
    pad_iloc = np.zeros(NCHUNK, np.int64)
    for ch in range(NCHUNK):
        cand = unocc_rows[(unocc_rows >= ch * CRE) & (unocc_rows < (ch + 1) * CRE)]
        assert len(cand) > 0, ch
        pad_iloc[ch] = cand[0] % CRE

    # --- per-core edge streams (no self-loops in stream) ------------------
    e_core = core_of[dst]
    e_w = w_of[dst]
    e_slot = s_of[dst].astype(np.float32)
    e_ch = cls[src]
    iloc = (row[src] % CRE).astype(np.int16)
    norm = (dinv[src] * dinv[dst]).astype(np.float32)

    key = (e_core * NCHUNK + e_ch) * WPC + e_w
    nk = NCORES * NCHUNK * WPC
    cnt = np.bincount(key, minlength=nk).reshape(NCORES, NCHUNK, WPC)
    G = np.ceil(cnt.max(axis=0) / 128.0).astype(np.int64)   # [NCHUNK, WPC]
    Gc = G.sum(axis=1)
    GT = int(G.sum())

    # emission order (w, ch, j); chunk-local gather order is (w, j)
    gbase = np.zeros((WPC, NCHUNK), np.int64)
    run = 0
    for w in range(WPC):
        for ch in range(NCHUNK):
            gbase[w, ch] = run
            run += G[ch, w]
    assert run == GT
    cbase = np.zeros((WPC, NCHUNK), np.int64)
    crun = np.zeros(NCHUNK, np.int64)
    for w in range(WPC):
        for ch in range(NCHUNK):
            cbase[w, ch] = crun[ch]
            crun[ch] += G[ch, w]
    assert (crun == Gc).all()

    eorder = np.lexsort((e_ch, e_w, e_core))
    key_s = ((e_core * WPC + e_w) * NCHUNK + e_ch)[eorder]
    iloc_s = iloc[eorder]
    slot_s = e_slot[eorder]
    norm_s = norm[eorder]
    bounds = np.searchsorted(key_s, np.arange(NCORES * WPC * NCHUNK + 1))

    mg = max(int(c) for c in Gc) * 128
    idx_streams = np.zeros((NCORES, NCHUNK, mg), np.int16)
    for ch in range(NCHUNK):
        idx_streams[:, ch, :] = pad_iloc[ch]
    dloc2d = np.full((NCORES, 128, GT), -1.0, np.float32)
    norm2d = np.zeros((NCORES, 128, GT), np.float32)
    for c in range(NCORES):
        for w in range(WPC):
            for ch in range(NCHUNK):
                k = (c * WPC + w) * NCHUNK + ch
                b0, b1 = bounds[k], bounds[k + 1]
                n = b1 - b0
                g = int(G[ch, w])
                assert n <= g * 128
                co = int(cbase[w, ch]) * 128
                idx_streams[c, ch, co:co + n] = iloc_s[b0:b1]
                gg = int(gbase[w, ch])
                sl = np.full(g * 128, -1.0, np.float32)
                nv = np.zeros(g * 128, np.float32)
                sl[:n] = slot_s[b0:b1]
                nv[:n] = norm_s[b0:b1]
                dloc2d[c, :, gg:gg + g] = sl.reshape(g, 128).T
                norm2d[c, :, gg:gg + g] = nv.reshape(g, 128).T

    idx2d = np.zeros((NCORES, NCHUNK, 128, mg // 16), np.int16)
    for c in range(NCORES):
        for ch in range(NCHUNK):
            a = idx_streams[c, ch].reshape(-1, 16).T
            idx2d[c, ch] = np.tile(a, (8, 1))

    # --- per-core aux tensors --------------------------------------------
    nodes = np.arange(N)
    perm = np.full((NCORES, SH), -1, np.int64)    # col i = w*128+s -> node
    perm[core_of, w_of * WIN + s_of] = nodes

    dinv2 = np.zeros((NCORES, WIN, WPC), np.float32)
    dinv2[core_of, s_of, w_of] = (dinv[nodes] ** 2).astype(np.float32)

    counts = np.maximum(np.bincount(batch, minlength=NG), 1).astype(np.float64)
    g2d = np.zeros((NCORES, WIN, WPC * NG), np.float16)
    g2d[core_of, s_of, w_of * NG + batch[nodes]] = (
        1.0 / counts[batch[nodes]]).astype(np.float16)

    return dict(
        G=G, Gc=Gc, GT=GT, gbase=gbase, cbase=cbase,
        idx2d=idx2d, dloc2d=dloc2d, norm2d=norm2d,
        core_of=core_of, w_of=w_of, s_of=s_of, row=row, shrow=shrow,
        perm=perm, dinv2=dinv2, g2d=g2d, counts=counts,
        unocc_rows=np.sort(unocc_rows), pad_iloc=pad_iloc,
    )


# ----------------------------------------------------------------------------
# device program pieces
# ----------------------------------------------------------------------------
def _preload_idx(nc, tc, ctx, idx_aps, sched):
    Gc = sched["Gc"]
    idxc = ctx.enter_context(tc.tile_pool(name="idxc", bufs=1))
    idx_sb = {}
    for ch in range(NCHUNK):
        if Gc[ch] == 0:
            continue
        it = idxc.tile([128, int(Gc[ch]) * 8], I16, tag=f"idx{ch}")
        nc.sync.dma_start(it[:], idx_aps[ch])
        idx_sb[ch] = it
    return idx_sb


def _emit_agg(nc, tc, ctx, tab_ap, dloc_sb, norm_sb, iota_sb, iotacol_sb,
              dinv2_sb, own_sb, bias_sb, hT, sched, winps, idx_sb,
              on_window=None):
    """Aggregate edges + per-window self-loop diag matmul.
    hT[:, w*128:(w+1)*128] = relu(agg_w + bias), feature-major."""
    G, Gc = sched["G"], sched["Gc"]

    gath = {}
    for ch in range(NCHUNK):
        if Gc[ch] == 0:
            continue
        gath[ch] = ctx.enter_context(tc.tile_pool(name=f"gath{ch}", bufs=3))

    tiles = {ch: [] for ch in range(NCHUNK)}
    issued = {ch: 0 for ch in range(NCHUNK)}

    def ensure(ch, upto):
        while issued[ch] <= upto:
            g0 = issued[ch]
            rem = Gc[ch] - g0
            ng = int(min(GCALL if rem > 2 * GCALL else GCALL // 2, rem))
            gt = gath[ch].tile([128, GCALL * F], F16, tag="gt")
            base = ch * CRE
            nc.gpsimd.dma_gather(
                gt[:, :ng * F].rearrange("p (g e) -> p g e", e=F),
                tab_ap[base:base + CRE, :],
                idx_sb[ch][:, g0 * 8:(g0 + ng) * 8], ng * 128, ng * 128, F,
                single_packet=False,
            )
            tiles[ch].append((gt, g0, ng))
            issued[ch] += ng

    ohp = ctx.enter_context(tc.tile_pool(name="ohp", bufs=20))
    gbase, cbase = sched["gbase"], sched["cbase"]
    for w in range(WPC):
        wt = winps.tile([128, 128], F32, tag="win")
        # self-loop diag: oh[s, j] = (j == s) * dinv2[s, w]
        ohs = ohp.tile([128, 128], F16, tag="oh")
        nc.vector.tensor_scalar(
            ohs[:], iota_sb[:], iotacol_sb[:, 0:1], dinv2_sb[:, w:w + 1],
            ALU.is_equal, ALU.mult,
        )
        nc.tensor.matmul(wt[:], own_sb[:, w * F:(w + 1) * F], ohs[:],
                         start=True, stop=False)
        total = int(sum(G[ch, w] for ch in range(NCHUNK)))
        done = 0
        for ch in range(NCHUNK):
            g = int(G[ch, w])
            for j in range(g):
                cg = int(cbase[w, ch]) + j
                ensure(ch, cg)
                gt, g0, ng = next(
                    t for t in tiles[ch] if t[1] <= cg < t[1] + t[2])
                k = cg - g0
                gg = int(gbase[w, ch]) + j
                oh = ohp.tile([128, 128], F16, tag="oh")
                nc.vector.tensor_scalar(
                    oh[:], iota_sb, dloc_sb[:, gg:gg + 1],
                    norm_sb[:, gg:gg + 1], ALU.is_equal, ALU.mult,
                )
                done += 1
                nc.tensor.matmul(
                    wt[:], gt[:, k * F:(k + 1) * F], oh[:],
                    start=False, stop=(done == total),
                )
        nc.scalar.activation(hT[:, w * F:(w + 1) * F], wt[:], AF.Relu,
                             bias=bias_sb)
        if on_window is not None:
            on_window(w)


# ----------------------------------------------------------------------------
# builders
# ----------------------------------------------------------------------------
def _build_launch1():
    """Sharded dense D1: xTc [128, SH] -> g1s [SH, 128] (rows s*98+w)."""
    nc = bacc.Bacc("TRN2", target_bir_lowering=False, debug=False,
                   num_devices=NCORES)
    xT = nc.dram_tensor("xT", [128, SH], FP8, kind="ExternalInput")
    w1 = nc.dram_tensor("w1", [128, 128], F16, kind="ExternalInput")
    wc1 = nc.dram_tensor("wc1", [128, 128], F16, kind="ExternalInput")
    b1 = nc.dram_tensor("b1", [128, 1], F32, kind="ExternalInput")
    g1s = nc.dram_tensor("g1s", [SH, F], FP8, kind="ExternalOutput")

    with tile.TileContext(nc) as tc, contextlib.ExitStack() as ctx:
        const = ctx.enter_context(tc.tile_pool(name="const", bufs=1))
        w1_sb = const.tile([128, 128], F16, tag="w1")
        nc.sync.dma_start(w1_sb[:], w1.ap())
        wc1_sb = const.tile([128, 128], F16, tag="wc1")
        nc.sync.dma_start(wc1_sb[:], wc1.ap())
        b1_sb = const.tile([128, 1], F32, tag="b1")
        nc.sync.dma_start(b1_sb[:], b1.ap())

        mm1ps = ctx.enter_context(tc.tile_pool(name="mm1ps", bufs=2,
                                               space="PSUM"))
        tabps = ctx.enter_context(tc.tile_pool(name="tabps", bufs=2,
                                               space="PSUM"))
        hpool = ctx.enter_context(tc.tile_pool(name="hpool", bufs=3))
        stpool = ctx.enter_context(tc.tile_pool(name="stpool", bufs=3))
        xin = ctx.enter_context(tc.tile_pool(name="xin", bufs=3))

        g1s_pm = g1s.ap().rearrange("(s u) f -> s (u f)", s=128)
        xt_cache = {}

        def src1(u512):
            blk = u512 // 4
            if blk not in xt_cache:
                t = xin.tile([128, 2048], FP8, tag="xt")
                cw = min(2048, SH - blk * 2048)
                nc.scalar.dma_start(t[:, :cw], xT.ap()[:, blk * 2048:blk * 2048 + cw])
                xt_cache.clear()
                xt_cache[blk] = t
            return xt_cache[blk]

        NT = (SH + 511) // 512          # 25 tiles, last = 256 cols
        st = None
        for u512 in range(NT):
            c0 = u512 * 512
            cw = min(512, SH - c0)
            xt = src1(u512)
            xs = xt[:, (u512 % 4) * 512:(u512 % 4) * 512 + cw]
            p1 = mm1ps.tile([128, 512], F32, tag="p1")
            nc.tensor.matmul(p1[:, :cw], w1_sb[:], xs, start=True, stop=True)
            h1 = hpool.tile([128, 512], F16, tag="h1")
            nc.scalar.activation(h1[:, :cw], p1[:, :cw], AF.Relu,
                                 bias=b1_sb[:, 0:1])
            q8 = u512 % 2
            if q8 == 0:
                st = stpool.tile([128, 1024], FP8, tag="st")
                tp = tabps.tile([128, 1024], F32, tag="tp")
                _build_launch1.tp = tp
            tp = _build_launch1.tp
            for q in range(cw // 128):
                nc.tensor.matmul(
                    tp[:, (q8 * 4 + q) * 128:(q8 * 4 + q + 1) * 128],
                    h1[:, q * 128:(q + 1) * 128], wc1_sb[:],
                    start=True, stop=True,
                )
            if q8 == 1 or u512 == NT - 1:
                nu = q8 * 4 + cw // 128
                nc.vector.tensor_copy(st[:, :nu * 128], tp[:, :nu * 128])
                u0 = (u512 // 2) * 8
                nc.sync.dma_start(g1s_pm[:, u0 * 128:(u0 + nu) * 128],
                                  st[:, :nu * 128])

    nc.compile()
    return nc


def _build_launch2(prep):
    """Aggregate conv1 + fused D2 -> g2s shard rows."""
    nc = bacc.Bacc("TRN2", target_bir_lowering=False, debug=False,
                   num_devices=NCORES)
    GT = prep["GT"]
    Gc = prep["Gc"]

    tab1 = nc.dram_tensor("tab1", [TABR, F], F16, kind="ExternalInput")
    own1 = nc.dram_tensor("own1", [128, WPC * F], FP8, kind="ExternalInput")
    idxs = [nc.dram_tensor("idx%d" % ch, [128, int(Gc[ch]) * 8], I16,
                           kind="ExternalInput") for ch in range(NCHUNK)]
    dloc = nc.dram_tensor("dloc", [128, GT], F32, kind="ExternalInput")
    norm = nc.dram_tensor("norm", [128, GT], F32, kind="ExternalInput")
    # packed per-partition consts: dinv2 f32[98] | iotac f32 | bc1 f32 |
    # bfc2 f32 | iota f16[128] | wfc2 f16[128] | wc2 f16[128]  (1172 B)
    PKB = 1172
    pk = nc.dram_tensor("pk", [128, PKB], mybir.dt.uint8, kind="ExternalInput")
    g2s = nc.dram_tensor("g2s", [SH, F], FP8, kind="ExternalOutput")

    with tile.TileContext(nc) as tc, contextlib.ExitStack() as ctx:
        const = ctx.enter_context(tc.tile_pool(name="const", bufs=1))
        big = ctx.enter_context(tc.tile_pool(name="big", bufs=1))
        _n = [0]

        def ld(ap, shape, dtype):
            _n[0] += 1
            t = const.tile(shape, dtype, tag="c%d" % _n[0])
            nc.sync.dma_start(t[:], ap)
            return t

        pk_sb = ld(pk.ap(), [128, PKB], mybir.dt.uint8)
        pv = pk_sb[:]
        dinv2_sb = pv[:, 0:392].bitcast(F32)
        iotac_sb = pv[:, 392:396].bitcast(F32)
        bc1_sb = pv[:, 396:400].bitcast(F32)
        bfc2_sb = pv[:, 400:404].bitcast(F32)
        iota_sb = pv[:, 404:660].bitcast(F16)
        wfc2_sb = pv[:, 660:916].bitcast(F16)
        wc2_sb = pv[:, 916:1172].bitcast(F16)
        dloc_sb = ld(dloc.ap(), [128, GT], F32)
        norm_sb = ld(norm.ap(), [128, GT], F32)
        own_sb = big.tile([128, WPC * F], FP8, tag="own")
        nc.sync.dma_start(own_sb[:], own1.ap())
        idx_sb = _preload_idx(nc, tc, ctx, [a.ap() for a in idxs], prep)
        h2T = big.tile([128, SH], F16, tag="h2T")

        winps = ctx.enter_context(tc.tile_pool(name="winps", bufs=2,
                                               space="PSUM"))
        mm1ps = ctx.enter_context(tc.tile_pool(name="mm1ps", bufs=2,
                                               space="PSUM"))
        tabps = ctx.enter_context(tc.tile_pool(name="tabps", bufs=2,
                                               space="PSUM"))
        hpool = ctx.enter_context(tc.tile_pool(name="hpool", bufs=3))
        stpool = ctx.enter_context(tc.tile_pool(name="stpool", bufs=3))

        g2s_pm = g2s.ap().rearrange("(s u) f -> s (u f)", s=128)
        d2state = {}

        def d2_tile(w):
            if w % 4 != 3 and w != WPC - 1:
                return
            u512 = w // 4
            c0 = u512 * 512
            cw = min(512, SH - c0)
            p1 = mm1ps.tile([128, 512], F32, tag="p1")
            nc.tensor.matmul(p1[:, :cw], wfc2_sb, h2T[:, c0:c0 + cw],
                             start=True, stop=True)
            h1 = hpool.tile([128, 512], F16, tag="h1")
            nc.scalar.activation(h1[:, :cw], p1[:, :cw], AF.Relu,
                                 bias=bfc2_sb)
            tp2 = tabps.tile([128, 512], F32, tag="tp")
            for q in range(cw // 128):
                nc.tensor.matmul(
                    tp2[:, q * 128:(q + 1) * 128],
                    h1[:, q * 128:(q + 1) * 128], wc2_sb,
                    start=True, stop=True,
                )
            nu = cw // 128
            st = stpool.tile([128, 512], FP8, tag="st")
            nc.vector.tensor_copy(st[:, :nu * 128], tp2[:, :nu * 128])
            u0 = u512 * 4
            nc.sync.dma_start(g2s_pm[:, u0 * 128:(u0 + nu) * 128],
                              st[:, :nu * 128])

        _emit_agg(nc, tc, ctx, tab1.ap(), dloc_sb, norm_sb, iota_sb,
                  iotac_sb, dinv2_sb, own_sb, bc1_sb, h2T, prep, winps,
                  idx_sb, on_window=d2_tile)

    nc.compile()
    return nc


def _build_launch3(prep):
    """Aggregate conv2 + D3 + graph-pool partials."""
    nc = bacc.Bacc("TRN2", target_bir_lowering=False, debug=False,
                   num_devices=NCORES)
    GT = prep["GT"]
    Gc = prep["Gc"]

    tab2 = nc.dram_tensor("tab2", [TABR, F], F16, kind="ExternalInput")
    own2 = nc.dram_tensor("own2", [128, WPC * F], FP8, kind="ExternalInput")
    idxs = [nc.dram_tensor("idx%d" % ch, [128, int(Gc[ch]) * 8], I16,
                           kind="ExternalInput") for ch in range(NCHUNK)]
    dloc = nc.dram_tensor("dloc", [128, GT], F32, kind="ExternalInput")
    norm = nc.dram_tensor("norm", [128, GT], F32, kind="ExternalInput")
    # packed: dinv2 f32[98] | iotac f32 | bc2 f32 | iota f16[128] | wfc f16[64]
    PKB = 784
    pk = nc.dram_tensor("pk", [128, PKB], mybir.dt.uint8, kind="ExternalInput")
    g2d = nc.dram_tensor("g2d", [128, WPC * NG], F16, kind="ExternalInput")
    pool = nc.dram_tensor("pool", [NG, NOUT], F32, kind="ExternalOutput")

    with tile.TileContext(nc) as tc, contextlib.ExitStack() as ctx:
        const = ctx.enter_context(tc.tile_pool(name="const", bufs=1))
        big = ctx.enter_context(tc.tile_pool(name="big", bufs=1))
        _n = [0]

        def ld(ap, shape, dtype):
            _n[0] += 1
            t = const.tile(shape, dtype, tag="c%d" % _n[0])
            nc.sync.dma_start(t[:], ap)
            return t

        pk_sb = ld(pk.ap(), [128, PKB], mybir.dt.uint8)
        pv = pk_sb[:]
        dinv2_sb = pv[:, 0:392].bitcast(F32)
        iotac_sb = pv[:, 392:396].bitcast(F32)
        bc2_sb = pv[:, 396:400].bitcast(F32)
        iota_sb = pv[:, 400:656].bitcast(F16)
        wfc_sb = pv[:, 656:784].bitcast(F16)
        g2d_sb = ld(g2d.ap(), [128, WPC * NG], F16)
        dloc_sb = ld(dloc.ap(), [128, GT], F32)
        norm_sb = ld(norm.ap(), [128, GT], F32)
        own_sb = big.tile([128, WPC * F], FP8, tag="own")
        nc.sync.dma_start(own_sb[:], own2.ap())
        idx_sb = _preload_idx(nc, tc, ctx, [a.ap() for a in idxs], prep)
        h4T = big.tile([128, SH], F16, tag="h4T")

        winps = ctx.enter_context(tc.tile_pool(name="winps", bufs=2,
                                               space="PSUM"))
        psd = ctx.enter_context(tc.tile_pool(name="psd", bufs=3, space="PSUM"))
        osb = ctx.enter_context(tc.tile_pool(name="osb", bufs=4))
        psp = ctx.enter_context(tc.tile_pool(name="psp", bufs=1, space="PSUM"))
        poolps = psp.tile([NG, NOUT], F32)

        def d3_win(w):
            pd = psd.tile([128, NOUT], F32, tag="pd")
            nc.tensor.matmul(pd[:], h4T[:, w * F:(w + 1) * F], wfc_sb,
                             start=True, stop=True)
            ot = osb.tile([128, NOUT], F16, tag="ot")
            nc.scalar.activation(ot[:], pd[:], AF.Copy)
            nc.tensor.matmul(poolps[:], g2d_sb[:, w * NG:(w + 1) * NG],
                             ot[:], start=(w == 0), stop=(w == WPC - 1),
                             skip_group_check=True)

        _emit_agg(nc, tc, ctx, tab2.ap(), dloc_sb, norm_sb, iota_sb,
                  iotac_sb, dinv2_sb, own_sb, bc2_sb, h4T, prep, winps,
                  idx_sb, on_window=d3_win)

        pres = osb.tile([NG, NOUT], F32, tag="pres")
        nc.vector.tensor_copy(pres[:], poolps[:])
        nc.sync.dma_start(pool.ap(), pres[:])

    nc.compile()
    return nc


def _np16(x):
    return np.ascontiguousarray(x, np.float16)


def _tl_ns(nc):
    from concourse.timeline_sim import TimelineSim
    tl = TimelineSim(nc, trace=False)
    tl.simulate()
    return int(tl.time)


def kernel(x, src, dst, batch, W_fc1, b_fc1, W_c1, b_c1, W_fc2, b_fc2, W_c2,
           b_c2, W_fc, b_fc):
    global LAST_EXEC_NS, LAST_INFO
    x = np.asarray(x, np.float32)
    prep = _prep(src, dst, batch)
    trace = os.environ.get("KERNEL_TRACE", "0") == "1"
    timing = os.environ.get("KERNEL_TIME", "0") == "1"

    col = lambda b: np.ascontiguousarray(
        np.asarray(b, np.float32).reshape(-1, 1))
    iota = np.tile(np.arange(128, dtype=np.float16), (128, 1))
    iotac = np.arange(128, dtype=np.float32).reshape(128, 1)

    def _packed(arrs):
        return np.ascontiguousarray(np.concatenate(
            [np.ascontiguousarray(a).view(np.uint8).reshape(128, -1)
             for a in arrs], axis=1))

    # ---- launch 1: sharded dense -> g1s shards --------------------------
    nc1 = _build_launch1()
    in_maps1 = []
    for c in range(NCORES):
        import ml_dtypes
        pm = prep["perm"][c]
        xTc = np.zeros((SH, F), ml_dtypes.float8_e4m3fn)
        m = pm >= 0
        xTc[m] = x[pm[m]].astype(ml_dtypes.float8_e4m3fn)
        in_maps1.append({
            "xT": np.ascontiguousarray(xTc.T), "w1": _np16(W_fc1),
            "wc1": _np16(W_c1), "b1": col(b_fc1),
        })
    r1 = run_bass_kernel_spmd(nc1, in_maps1, core_ids=list(range(NCORES)),
                              trace=trace)
    t1 = r1.exec_time_ns or (_tl_ns(nc1) if timing else None)

    # host: assemble table1 + own views
    g1 = [np.asarray(r1.results[c]["g1s"]) for c in range(NCORES)]
    tab1 = np.zeros((TABR, F), np.float16)
    t1v = tab1.reshape(NCHUNK, NCORES, CSH, F)
    for c in range(NCORES):
        t1v[:, c] = g1[c].reshape(NCHUNK, CSH, F).astype(np.float16)
    tab1[prep["unocc_rows"]] = 0.0

    def own_view(gs):
        return np.ascontiguousarray(gs.reshape(128, WPC * F))

    def im_agg(c, tabname, tabv, ownv, extra):
        im = {
            tabname: tabv, "dloc": prep["dloc2d"][c], "norm": prep["norm2d"][c],
        }
        for ch in range(NCHUNK):
            gc = int(prep["Gc"][ch]) * 8
            im["idx%d" % ch] = np.ascontiguousarray(
                prep["idx2d"][c, ch][:, :gc])
        im.update(extra)
        im["own1" if tabname == "tab1" else "own2"] = ownv
        return im

    # ---- launch 2: agg conv1 + D2 -> g2s shards -------------------------
    nc2 = _build_launch2(prep)
    in_maps2 = []
    for c in range(NCORES):
        pk2 = _packed([
            np.ascontiguousarray(prep["dinv2"][c], np.float32),
            iotac,
            np.asarray(b_c1, np.float32).reshape(128, 1),
            np.asarray(b_fc2, np.float32).reshape(128, 1),
            iota, _np16(W_fc2), _np16(W_c2),
        ])
        in_maps2.append(im_agg(c, "tab1", tab1, own_view(g1[c]), {"pk": pk2}))
    r2 = run_bass_kernel_spmd(nc2, in_maps2, core_ids=list(range(NCORES)),
                              trace=trace)
    t2 = r2.exec_time_ns or (_tl_ns(nc2) if timing else None)

    g2 = [np.asarray(r2.results[c]["g2s"]) for c in range(NCORES)]
    tab2 = np.zeros((TABR, F), np.float16)
    t2v = tab2.reshape(NCHUNK, NCORES, CSH, F)
    for c in range(NCORES):
        t2v[:, c] = g2[c].reshape(NCHUNK, CSH, F).astype(np.float16)
    tab2[prep["unocc_rows"]] = 0.0

    # ---- launch 3: agg conv2 + D3 + pool --------------------------------
    nc3 = _build_launch3(prep)
    wfcp = np.zeros((128, NOUT), np.float16)
    wfcp[:] = _np16(W_fc)
    in_maps3 = []
    for c in range(NCORES):
        pk3 = _packed([
            np.ascontiguousarray(prep["dinv2"][c], np.float32),
            iotac,
            np.asarray(b_c2, np.float32).reshape(128, 1),
            iota, wfcp,
        ])
        in_maps3.append(im_agg(c, "tab2", tab2, own_view(g2[c]), {
            "pk": pk3, "g2d": np.ascontiguousarray(prep["g2d"][c])}))
    r3 = run_bass_kernel_spmd(nc3, in_maps3, core_ids=list(range(NCORES)),
                              trace=trace)
    t3 = r3.exec_time_ns or (_tl_ns(nc3) if timing else None)

    out = np.zeros((NG, NOUT), np.float64)
    for c in range(NCORES):
        out += np.asarray(r3.results[c]["pool"]).astype(np.float64)
    out = out + np.asarray(b_fc, np.float64)[None, :]

    LAST_EXEC_NS = (t1 or 0) + (t2 or 0) + (t3 or 0)
    LAST_INFO = {"t1": t1, "t2": t2, "t3": t3, "GT": prep["GT"]}
    return out.astype(np.float32)


# revision 11
# speedup vs baseline: 1.3268x; 1.0270x over previous
"""2-layer GCN + dense layers + mean-pool on 8 trn2 NeuronCores (Bass/Tile).

v3 design (3 launches, sharded dense, self-loops via local diag matmuls).

GCNConv out = D^-1/2 (A+I) D^-1/2 (h W) + b factorizes as
  table[v]  = (h W)[v]                      (unscaled, per node)
  agg[d]    = sum_{e: dst=d} norm_e * table[src_e] + dinv_d^2 * table[d]
  h2[d]     = relu(agg[d] + b)
Self-loop terms never enter the gather stream: each core keeps its own dst
shard's table rows (slot-major, contiguous) in SBUF and adds them with one
matmul per window against a DVE-built diagonal (values dinv_d^2).

Node placement: greedy binning assigns each node to a (core, window) bin
(128 slots each) with per-chunk caps, where chunk class = node_id % 4 and
slots are class quarters (slot = class*32 + sloc).  Table row of node v:
  row(v) = class*25088 + core*3136 + sloc*98 + w   (= core shard row s*98+w)
so each core's dense output shard [12544, 128] is contiguous, gather chunks
(int16 idx) are fixed row ranges, and host-side assembly is pure reshape.

Launch 1: per-core dense D1 (x shard -> relu(xW1+b1) Wc1) -> shard rows.
Host: assemble table1, build own-shard views.  Launch 2: aggregate conv1
(dma_gather per 128-edge group + one-hot matmul, feature-major windows),
fused D2 -> table2 shard rows.  Host: assemble table2.  Launch 3: aggregate
conv2 + D3 + graph-pool partials; host sums partials + b_fc.

Both conv layers share one gather schedule (same graph, same row map).
"""

import os
import sys

sys.path.insert(0, "/opt/trn_rl_repo")

import contextlib

import numpy as np

import concourse.bass as bass
import concourse.tile as tile
from concourse import bacc, mybir
from concourse.bass_utils import run_bass_kernel_spmd

F32 = mybir.dt.float32
F16 = mybir.dt.float16
I16 = mybir.dt.int16
FP8 = mybir.dt.float8e4
AF = mybir.ActivationFunctionType
ALU = mybir.AluOpType

N = 100000
F = 128
NOUT = 64
NG = 64
NCORES = 8
WPC = 98                    # windows per core
WIN = 128
SH = WPC * WIN              # 12544 shard rows per core
NBINS = NCORES * WPC
NCHUNK = 4
CSH = SH // NCHUNK          # 3136 rows per (core, class)
CRE = NCORES * CSH          # 25088 rows per gather chunk
TABR = NCHUNK * CRE         # 100352 table rows
GCALL = 16                  # gather groups per dma_gather call

LAST_EXEC_NS = None
LAST_INFO = {}


# ----------------------------------------------------------------------------
# host-side graph prep
# ----------------------------------------------------------------------------
def _prep(src, dst, batch):
    src = np.asarray(src, np.int64)
    dst = np.asarray(dst, np.int64)
    batch = np.asarray(batch, np.int64)

    deg = np.bincount(dst, minlength=N).astype(np.float64) + 1.0
    dinv = 1.0 / np.sqrt(deg)

    cls = np.arange(N, dtype=np.int64) % NCHUNK
    k4 = np.zeros((N, NCHUNK), np.int64)
    np.add.at(k4, (dst, cls[src]), 1)
    ktot = k4.sum(1)

    # --- greedy binning: (core, window) bins, class quotas of 32 ----------
    def _greedy(margin, zadd):
        capG = {4: 4 * 128 - margin, 5: 5 * 128 - margin, 6: 6 * 128 - margin}
        share = k4.sum(0).max() / max(ktot.sum(), 1)
        Ecore = ktot.sum() / NCORES * 1.004
        Ty = capG[4] / share
        Tz = capG[5] / share
        z = int(np.ceil(max(0.0, (Ecore - WPC * Ty) / (Tz - Ty)))) + zadd
        z = min(max(z, 0), WPC)
        wclass = np.array([5] * z + [4] * (WPC - z))
        caps = np.zeros((NBINS, NCHUNK), np.float64)
        for b in range(NBINS):
            caps[b, :] = capG[wclass[b % WPC]]
        rem = caps.copy()
        mu = caps / 128.0
        slots = np.full(NBINS, 128, np.float64)
        clsroom = np.full((NBINS, NCHUNK), 32, np.int64)
        rng = np.random.default_rng(0)
        order = rng.permutation(N)
        bin_of = np.full(N, -1, np.int64)
        for v in order:
            need = k4[v]
            cl = cls[v]
            ok = (rem >= need).all(1) & (slots > 0) & (clsroom[:, cl] > 0)
            if not ok.any():
                ok = (slots > 0) & (clsroom[:, cl] > 0)
                if not ok.any():
                    ok = clsroom[:, cl] > 0
            dev = rem - need - (slots[:, None] - 1) * mu
            sc = np.where(ok, (dev * dev).sum(1), np.inf)
            b = int(np.argmin(sc))
            bin_of[v] = b
            rem[b] -= need
            slots[b] -= 1
            clsroom[b, cl] -= 1
        cntb = np.zeros((NBINS, NCHUNK), np.int64)
        for ch in range(NCHUNK):
            np.add.at(cntb[:, ch], bin_of, k4[:, ch])
        Gb = np.ceil(cntb.reshape(NCORES, WPC, NCHUNK).max(0) / 128.0)
        return bin_of, int(Gb.sum())

    best = None
    for margin, zadd in ((2, -1), (4, -1), (12, 0)):
        bo, gt = _greedy(margin, zadd)
        if best is None or gt < best[1]:
            best = (bo, gt)
        if gt <= 1576:
            break
    bin_of = best[0]

    core_of = bin_of // WPC
    w_of = bin_of % WPC

    # slots: class quarters; sloc = running fill per (bin, class)
    sloc_of = np.empty(N, np.int64)
    fill = np.zeros((NBINS, NCHUNK), np.int64)
    sidx = np.argsort(bin_of, kind="stable")
    for v in sidx:
        b, cl = bin_of[v], cls[v]
        sloc_of[v] = fill[b, cl]
        fill[b, cl] += 1
    assert fill.max() <= 32
    s_of = cls * 32 + sloc_of                     # global slot 0..127
    row = cls * CRE + core_of * CSH + sloc_of * 98 + w_of   # table row
    shrow = s_of * WPC + w_of                     # shard-local row

    # unoccupied (c, w, s) slots -> zero rows / pad gather targets
    occ = np.zeros((NCORES, WPC, WIN), bool)
    occ[core_of, w_of, s_of] = True
    uc, uw, us = np.nonzero(~occ)
    unocc_rows = ((us // 32) * CRE + uc * CSH + (us % 32) * 98 + uw)
    pad_iloc = np.zeros(NCHUNK, np.int64)
    for ch in range(NCHUNK):
        cand = unocc_rows[(unocc_rows >= ch * CRE) & (unocc_rows < (ch + 1) * CRE)]
        assert len(cand) > 0, ch
        pad_iloc[ch] = cand[0] % CRE

    # --- per-core edge streams (no self-loops in stream) ------------------
    e_core = core_of[dst]
    e_w = w_of[dst]
    e_slot = s_of[dst].astype(np.float32)
    e_ch = cls[src]
    iloc = (row[src] % CRE).astype(np.int16)
    norm = (dinv[src] * dinv[dst]).astype(np.float32)

    key = (e_core * NCHUNK + e_ch) * WPC + e_w
    nk = NCORES * NCHUNK * WPC
    cnt = np.bincount(key, minlength=nk).reshape(NCORES, NCHUNK, WPC)
    G = np.ceil(cnt.max(axis=0) / 128.0).astype(np.int64)   # [NCHUNK, WPC]
    Gc = G.sum(axis=1)
    GT = int(G.sum())

    # emission order (w, ch, j); chunk-local gather order is (w, j)
    gbase = np.zeros((WPC, NCHUNK), np.int64)
    run = 0
    for w in range(WPC):
        for ch in range(NCHUNK):
            gbase[w, ch] = run
            run += G[ch, w]
    assert run == GT
    cbase = np.zeros((WPC, NCHUNK), np.int64)
    crun = np.zeros(NCHUNK, np.int64)
    for w in range(WPC):
        for ch in range(NCHUNK):
            cbase[w, ch] = crun[ch]
            crun[ch] += G[ch, w]
    assert (crun == Gc).all()

    eorder = np.lexsort((e_ch, e_w, e_core))
    key_s = ((e_core * WPC + e_w) * NCHUNK + e_ch)[eorder]
    iloc_s = iloc[eorder]
    slot_s = e_slot[eorder]
    norm_s = norm[eorder]
    bounds = np.searchsorted(key_s, np.arange(NCORES * WPC * NCHUNK + 1))

    mg = max(int(c) for c in Gc) * 128
    idx_streams = np.zeros((NCORES, NCHUNK, mg), np.int16)
    for ch in range(NCHUNK):
        idx_streams[:, ch, :] = pad_iloc[ch]
    dloc2d = np.full((NCORES, 128, GT), -1.0, np.float32)
    norm2d = np.zeros((NCORES, 128, GT), np.float32)
    for c in range(NCORES):
        for w in range(WPC):
            for ch in range(NCHUNK):
                k = (c * WPC + w) * NCHUNK + ch
                b0, b1 = bounds[k], bounds[k + 1]
                n = b1 - b0
                g = int(G[ch, w])
                assert n <= g * 128
                co = int(cbase[w, ch]) * 128
                idx_streams[c, ch, co:co + n] = iloc_s[b0:b1]
                gg = int(gbase[w, ch])
                sl = np.full(g * 128, -1.0, np.float32)
                nv = np.zeros(g * 128, np.float32)
                sl[:n] = slot_s[b0:b1]
                nv[:n] = norm_s[b0:b1]
                dloc2d[c, :, gg:gg + g] = sl.reshape(g, 128).T
                norm2d[c, :, gg:gg + g] = nv.reshape(g, 128).T

    idx2d = np.zeros((NCORES, NCHUNK, 128, mg // 16), np.int16)
    for c in range(NCORES):
        for ch in range(NCHUNK):
            a = idx_streams[c, ch].reshape(-1, 16).T
            idx2d[c, ch] = np.tile(a, (8, 1))

    # --- per-core aux tensors --------------------------------------------
    nodes = np.arange(N)
    perm = np.full((NCORES, SH), -1, np.int64)    # col i = w*128+s -> node
    perm[core_of, w_of * WIN + s_of] = nodes

    dinv2 = np.zeros((NCORES, WIN, WPC), np.float32)
    dinv2[core_of, s_of, w_of] = (dinv[nodes] ** 2).astype(np.float32)

    counts = np.maximum(np.bincount(batch, minlength=NG), 1).astype(np.float64)
    g2d = np.zeros((NCORES, WIN, WPC * NG), np.float16)
    g2d[core_of, s_of, w_of * NG + batch[nodes]] = (
        1.0 / counts[batch[nodes]]).astype(np.float16)

    return dict(
        G=G, Gc=Gc, GT=GT, gbase=gbase, cbase=cbase,
        idx2d=idx2d, dloc2d=dloc2d, norm2d=norm2d,
        core_of=core_of, w_of=w_of, s_of=s_of, row=row, shrow=shrow,
        perm=perm, dinv2=dinv2, g2d=g2d, counts=counts,
        unocc_rows=np.sort(unocc_rows), pad_iloc=pad_iloc,
    )


# ----------------------------------------------------------------------------
# device program pieces
# ----------------------------------------------------------------------------
def _preload_idx(nc, tc, ctx, idx_aps, sched):
    Gc = sched["Gc"]
    idxc = ctx.enter_context(tc.tile_pool(name="idxc", bufs=1))
    idx_sb = {}
    for ch in range(NCHUNK):
        if Gc[ch] == 0:
            continue
        it = idxc.tile([128, int(Gc[ch]) * 8], I16, tag=f"idx{ch}")
        nc.sync.dma_start(it[:], idx_aps[ch])
        idx_sb[ch] = it
    return idx_sb


def _emit_agg(nc, tc, ctx, tab_ap, dloc_sb, norm_sb, iota_sb, iotacol_sb,
              dinv2_sb, own_sb, bias_sb, hT, sched, winps, idx_sb,
              on_window=None):
    """Aggregate edges + per-window self-loop diag matmul.
    hT[:, w*128:(w+1)*128] = relu(agg_w + bias), feature-major."""
    G, Gc = sched["G"], sched["Gc"]

    gath = {}
    for ch in range(NCHUNK):
        if Gc[ch] == 0:
            continue
        gath[ch] = ctx.enter_context(tc.tile_pool(name=f"gath{ch}", bufs=4))

    tiles = {ch: [] for ch in range(NCHUNK)}
    issued = {ch: 0 for ch in range(NCHUNK)}

    def ensure(ch, upto):
        while issued[ch] <= upto:
            g0 = issued[ch]
            rem = Gc[ch] - g0
            ng = int(min(GCALL if rem > 2 * GCALL else GCALL // 2, rem))
            gt = gath[ch].tile([128, GCALL * F], F16, tag="gt")
            base = ch * CRE
            nc.gpsimd.dma_gather(
                gt[:, :ng * F].rearrange("p (g e) -> p g e", e=F),
                tab_ap[base:base + CRE, :],
                idx_sb[ch][:, g0 * 8:(g0 + ng) * 8], ng * 128, ng * 128, F,
                single_packet=False,
            )
            tiles[ch].append((gt, g0, ng))
            issued[ch] += ng

    ohp = ctx.enter_context(tc.tile_pool(name="ohp", bufs=20))
    gbase, cbase = sched["gbase"], sched["cbase"]
    for w in range(WPC):
        wt = winps.tile([128, 128], F32, tag="win")
        # self-loop diag: oh[s, j] = (j == s) * dinv2[s, w]
        ohs = ohp.tile([128, 128], F16, tag="oh")
        nc.vector.tensor_scalar(
            ohs[:], iota_sb[:], iotacol_sb[:, 0:1], dinv2_sb[:, w:w + 1],
            ALU.is_equal, ALU.mult,
        )
        nc.tensor.matmul(wt[:], own_sb[:, w * F:(w + 1) * F], ohs[:],
                         start=True, stop=False)
        total = int(sum(G[ch, w] for ch in range(NCHUNK)))
        done = 0
        for ch in range(NCHUNK):
            g = int(G[ch, w])
            for j in range(g):
                cg = int(cbase[w, ch]) + j
                ensure(ch, cg)
                gt, g0, ng = next(
                    t for t in tiles[ch] if t[1] <= cg < t[1] + t[2])
                k = cg - g0
                gg = int(gbase[w, ch]) + j
                oh = ohp.tile([128, 128], F16, tag="oh")
                nc.vector.tensor_scalar(
                    oh[:], iota_sb, dloc_sb[:, gg:gg + 1],
                    norm_sb[:, gg:gg + 1], ALU.is_equal, ALU.mult,
                )
                done += 1
                nc.tensor.matmul(
                    wt[:], gt[:, k * F:(k + 1) * F], oh[:],
                    start=False, stop=(done == total),
                )
        nc.scalar.activation(hT[:, w * F:(w + 1) * F], wt[:], AF.Relu,
                             bias=bias_sb)
        if on_window is not None:
            on_window(w)


# ----------------------------------------------------------------------------
# builders
# ----------------------------------------------------------------------------
def _build_launch1():
    """Sharded dense D1: xTc [128, SH] -> g1s [SH, 128] (rows s*98+w)."""
    nc = bacc.Bacc("TRN2", target_bir_lowering=False, debug=False,
                   num_devices=NCORES)
    xT = nc.dram_tensor("xT", [128, SH], FP8, kind="ExternalInput")
    w1 = nc.dram_tensor("w1", [128, 128], F16, kind="ExternalInput")
    wc1 = nc.dram_tensor("wc1", [128, 128], F16, kind="ExternalInput")
    b1 = nc.dram_tensor("b1", [128, 1], F32, kind="ExternalInput")
    g1s = nc.dram_tensor("g1s", [SH, F], FP8, kind="ExternalOutput")

    with tile.TileContext(nc) as tc, contextlib.ExitStack() as ctx:
        const = ctx.enter_context(tc.tile_pool(name="const", bufs=1))
        w1_sb = const.tile([128, 128], F16, tag="w1")
        nc.sync.dma_start(w1_sb[:], w1.ap())
        wc1_sb = const.tile([128, 128], F16, tag="wc1")
        nc.sync.dma_start(wc1_sb[:], wc1.ap())
        b1_sb = const.tile([128, 1], F32, tag="b1")
        nc.sync.dma_start(b1_sb[:], b1.ap())

        mm1ps = ctx.enter_context(tc.tile_pool(name="mm1ps", bufs=2,
                                               space="PSUM"))
        tabps = ctx.enter_context(tc.tile_pool(name="tabps", bufs=2,
                                               space="PSUM"))
        hpool = ctx.enter_context(tc.tile_pool(name="hpool", bufs=3))
        stpool = ctx.enter_context(tc.tile_pool(name="stpool", bufs=3))
        xin = ctx.enter_context(tc.tile_pool(name="xin", bufs=3))

        g1s_pm = g1s.ap().rearrange("(s u) f -> s (u f)", s=128)
        xt_cache = {}

        def src1(u512):
            blk = u512 // 4
            if blk not in xt_cache:
                t = xin.tile([128, 2048], FP8, tag="xt")
                cw = min(2048, SH - blk * 2048)
                nc.scalar.dma_start(t[:, :cw], xT.ap()[:, blk * 2048:blk * 2048 + cw])
                xt_cache.clear()
                xt_cache[blk] = t
            return xt_cache[blk]

        NT = (SH + 511) // 512          # 25 tiles, last = 256 cols
        st = None
        for u512 in range(NT):
            c0 = u512 * 512
            cw = min(512, SH - c0)
            xt = src1(u512)
            xs = xt[:, (u512 % 4) * 512:(u512 % 4) * 512 + cw]
            p1 = mm1ps.tile([128, 512], F32, tag="p1")
            nc.tensor.matmul(p1[:, :cw], w1_sb[:], xs, start=True, stop=True)
            h1 = hpool.tile([128, 512], F16, tag="h1")
            nc.scalar.activation(h1[:, :cw], p1[:, :cw], AF.Relu,
                                 bias=b1_sb[:, 0:1])
            q8 = u512 % 2
            if q8 == 0:
                st = stpool.tile([128, 1024], FP8, tag="st")
                tp = tabps.tile([128, 1024], F32, tag="tp")
                _build_launch1.tp = tp
            tp = _build_launch1.tp
            for q in range(cw // 128):
                nc.tensor.matmul(
                    tp[:, (q8 * 4 + q) * 128:(q8 * 4 + q + 1) * 128],
                    h1[:, q * 128:(q + 1) * 128], wc1_sb[:],
                    start=True, stop=True,
                )
            if q8 == 1 or u512 == NT - 1:
                nu = q8 * 4 + cw // 128
                nc.vector.tensor_copy(st[:, :nu * 128], tp[:, :nu * 128])
                u0 = (u512 // 2) * 8
                nc.sync.dma_start(g1s_pm[:, u0 * 128:(u0 + nu) * 128],
                                  st[:, :nu * 128])

    nc.compile()
    return nc


def _build_launch2(prep):
    """Aggregate conv1 + fused D2 -> g2s shard rows."""
    nc = bacc.Bacc("TRN2", target_bir_lowering=False, debug=False,
                   num_devices=NCORES)
    GT = prep["GT"]
    Gc = prep["Gc"]

    tab1 = nc.dram_tensor("tab1", [TABR, F], F16, kind="ExternalInput")
    own1 = nc.dram_tensor("own1", [128, WPC * F], FP8, kind="ExternalInput")
    idxs = [nc.dram_tensor("idx%d" % ch, [128, int(Gc[ch]) * 8], I16,
                           kind="ExternalInput") for ch in range(NCHUNK)]
    dloc = nc.dram_tensor("dloc", [128, GT], F32, kind="ExternalInput")
    norm = nc.dram_tensor("norm", [128, GT], F32, kind="ExternalInput")
    # packed per-partition consts: dinv2 f32[98] | iotac f32 | bc1 f32 |
    # bfc2 f32 | iota f16[128] | wfc2 f16[128] | wc2 f16[128]  (1172 B)
    PKB = 1172
    pk = nc.dram_tensor("pk", [128, PKB], mybir.dt.uint8, kind="ExternalInput")
    g2s = nc.dram_tensor("g2s", [SH, F], FP8, kind="ExternalOutput")

    with tile.TileContext(nc) as tc, contextlib.ExitStack() as ctx:
        const = ctx.enter_context(tc.tile_pool(name="const", bufs=1))
        big = ctx.enter_context(tc.tile_pool(name="big", bufs=1))
        _n = [0]

        def ld(ap, shape, dtype):
            _n[0] += 1
            t = const.tile(shape, dtype, tag="c%d" % _n[0])
            nc.sync.dma_start(t[:], ap)
            return t

        pk_sb = ld(pk.ap(), [128, PKB], mybir.dt.uint8)
        pv = pk_sb[:]
        dinv2_sb = pv[:, 0:392].bitcast(F32)
        iotac_sb = pv[:, 392:396].bitcast(F32)
        bc1_sb = pv[:, 396:400].bitcast(F32)
        bfc2_sb = pv[:, 400:404].bitcast(F32)
        iota_sb = pv[:, 404:660].bitcast(F16)
        wfc2_sb = pv[:, 660:916].bitcast(F16)
        wc2_sb = pv[:, 916:1172].bitcast(F16)
        dloc_sb = ld(dloc.ap(), [128, GT], F32)
        norm_sb = ld(norm.ap(), [128, GT], F32)
        own_sb = big.tile([128, WPC * F], FP8, tag="own")
        nc.sync.dma_start(own_sb[:], own1.ap())
        idx_sb = _preload_idx(nc, tc, ctx, [a.ap() for a in idxs], prep)
        h2T = big.tile([128, SH], F16, tag="h2T")

        winps = ctx.enter_context(tc.tile_pool(name="winps", bufs=2,
                                               space="PSUM"))
        mm1ps = ctx.enter_context(tc.tile_pool(name="mm1ps", bufs=2,
                                               space="PSUM"))
        tabps = ctx.enter_context(tc.tile_pool(name="tabps", bufs=2,
                                               space="PSUM"))
        hpool = ctx.enter_context(tc.tile_pool(name="hpool", bufs=3))
        stpool = ctx.enter_context(tc.tile_pool(name="stpool", bufs=3))

        g2s_pm = g2s.ap().rearrange("(s u) f -> s (u f)", s=128)
        d2state = {}

        def d2_tile(w):
            if w % 4 != 3 and w != WPC - 1:
                return
            u512 = w // 4
            c0 = u512 * 512
            cw = min(512, SH - c0)
            p1 = mm1ps.tile([128, 512], F32, tag="p1")
            nc.tensor.matmul(p1[:, :cw], wfc2_sb, h2T[:, c0:c0 + cw],
                             start=True, stop=True)
            h1 = hpool.tile([128, 512], F16, tag="h1")
            nc.scalar.activation(h1[:, :cw], p1[:, :cw], AF.Relu,
                                 bias=bfc2_sb)
            tp2 = tabps.tile([128, 512], F32, tag="tp")
            for q in range(cw // 128):
                nc.tensor.matmul(
                    tp2[:, q * 128:(q + 1) * 128],
                    h1[:, q * 128:(q + 1) * 128], wc2_sb,
                    start=True, stop=True,
                )
            nu = cw // 128
            st = stpool.tile([128, 512], FP8, tag="st")
            nc.vector.tensor_copy(st[:, :nu * 128], tp2[:, :nu * 128])
            u0 = u512 * 4
            nc.sync.dma_start(g2s_pm[:, u0 * 128:(u0 + nu) * 128],
                              st[:, :nu * 128])

        _emit_agg(nc, tc, ctx, tab1.ap(), dloc_sb, norm_sb, iota_sb,
                  iotac_sb, dinv2_sb, own_sb, bc1_sb, h2T, prep, winps,
                  idx_sb, on_window=d2_tile)

    nc.compile()
    return nc


def _build_launch3(prep):
    """Aggregate conv2 + D3 + graph-pool partials."""
    nc = bacc.Bacc("TRN2", target_bir_lowering=False, debug=False,
                   num_devices=NCORES)
    GT = prep["GT"]
    Gc = prep["Gc"]

    tab2 = nc.dram_tensor("tab2", [TABR, F], F16, kind="ExternalInput")
    own2 = nc.dram_tensor("own2", [128, WPC * F], FP8, kind="ExternalInput")
    idxs = [nc.dram_tensor("idx%d" % ch, [128, int(Gc[ch]) * 8], I16,
                           kind="ExternalInput") for ch in range(NCHUNK)]
    dloc = nc.dram_tensor("dloc", [128, GT], F32, kind="ExternalInput")
    norm = nc.dram_tensor("norm", [128, GT], F32, kind="ExternalInput")
    # packed: dinv2 f32[98] | iotac f32 | bc2 f32 | iota f16[128] | wfc f16[64]
    PKB = 784
    pk = nc.dram_tensor("pk", [128, PKB], mybir.dt.uint8, kind="ExternalInput")
    g2d = nc.dram_tensor("g2d", [128, WPC * NG], F16, kind="ExternalInput")
    pool = nc.dram_tensor("pool", [NG, NOUT], F32, kind="ExternalOutput")

    with tile.TileContext(nc) as tc, contextlib.ExitStack() as ctx:
        const = ctx.enter_context(tc.tile_pool(name="const", bufs=1))
        big = ctx.enter_context(tc.tile_pool(name="big", bufs=1))
        _n = [0]

        def ld(ap, shape, dtype):
            _n[0] += 1
            t = const.tile(shape, dtype, tag="c%d" % _n[0])
            nc.sync.dma_start(t[:], ap)
            return t

        pk_sb = ld(pk.ap(), [128, PKB], mybir.dt.uint8)
        pv = pk_sb[:]
        dinv2_sb = pv[:, 0:392].bitcast(F32)
        iotac_sb = pv[:, 392:396].bitcast(F32)
        bc2_sb = pv[:, 396:400].bitcast(F32)
        iota_sb = pv[:, 400:656].bitcast(F16)
        wfc_sb = pv[:, 656:784].bitcast(F16)
        g2d_sb = ld(g2d.ap(), [128, WPC * NG], F16)
        dloc_sb = ld(dloc.ap(), [128, GT], F32)
        norm_sb = ld(norm.ap(), [128, GT], F32)
        own_sb = big.tile([128, WPC * F], FP8, tag="own")
        nc.sync.dma_start(own_sb[:], own2.ap())
        idx_sb = _preload_idx(nc, tc, ctx, [a.ap() for a in idxs], prep)
        h4T = big.tile([128, SH], F16, tag="h4T")

        winps = ctx.enter_context(tc.tile_pool(name="winps", bufs=2,
                                               space="PSUM"))
        psd = ctx.enter_context(tc.tile_pool(name="psd", bufs=3, space="PSUM"))
        osb = ctx.enter_context(tc.tile_pool(name="osb", bufs=4))
        psp = ctx.enter_context(tc.tile_pool(name="psp", bufs=1, space="PSUM"))
        poolps = psp.tile([NG, NOUT], F32)

        def d3_win(w):
            pd = psd.tile([128, NOUT], F32, tag="pd")
            nc.tensor.matmul(pd[:], h4T[:, w * F:(w + 1) * F], wfc_sb,
                             start=True, stop=True)
            ot = osb.tile([128, NOUT], F16, tag="ot")
            nc.scalar.activation(ot[:], pd[:], AF.Copy)
            nc.tensor.matmul(poolps[:], g2d_sb[:, w * NG:(w + 1) * NG],
                             ot[:], start=(w == 0), stop=(w == WPC - 1),
                             skip_group_check=True)

        _emit_agg(nc, tc, ctx, tab2.ap(), dloc_sb, norm_sb, iota_sb,
                  iotac_sb, dinv2_sb, own_sb, bc2_sb, h4T, prep, winps,
                  idx_sb, on_window=d3_win)

        pres = osb.tile([NG, NOUT], F32, tag="pres")
        nc.vector.tensor_copy(pres[:], poolps[:])
        nc.sync.dma_start(pool.ap(), pres[:])

    nc.compile()
    return nc


def _np16(x):
    return np.ascontiguousarray(x, np.float16)


def _tl_ns(nc):
    from concourse.timeline_sim import TimelineSim
    tl = TimelineSim(nc, trace=False)
    tl.simulate()
    return int(tl.time)


def kernel(x, src, dst, batch, W_fc1, b_fc1, W_c1, b_c1, W_fc2, b_fc2, W_c2,
           b_c2, W_fc, b_fc):
    global LAST_EXEC_NS, LAST_INFO
    x = np.asarray(x, np.float32)
    prep = _prep(src, dst, batch)
    trace = os.environ.get("KERNEL_TRACE", "0") == "1"
    timing = os.environ.get("KERNEL_TIME", "0") == "1"

    col = lambda b: np.ascontiguousarray(
        np.asarray(b, np.float32).reshape(-1, 1))
    iota = np.tile(np.arange(128, dtype=np.float16), (128, 1))
    iotac = np.arange(128, dtype=np.float32).reshape(128, 1)

    def _packed(arrs):
        return np.ascontiguousarray(np.concatenate(
            [np.ascontiguousarray(a).view(np.uint8).reshape(128, -1)
             for a in arrs], axis=1))

    # ---- launch 1: sharded dense -> g1s shards --------------------------
    nc1 = _build_launch1()
    in_maps1 = []
    for c in range(NCORES):
        import ml_dtypes
        pm = prep["perm"][c]
        xTc = np.zeros((SH, F), ml_dtypes.float8_e4m3fn)
        m = pm >= 0
        xTc[m] = x[pm[m]].astype(ml_dtypes.float8_e4m3fn)
        in_maps1.append({
            "xT": np.ascontiguousarray(xTc.T), "w1": _np16(W_fc1),
            "wc1": _np16(W_c1), "b1": col(b_fc1),
        })
    r1 = run_bass_kernel_spmd(nc1, in_maps1, core_ids=list(range(NCORES)),
                              trace=trace)
    t1 = r1.exec_time_ns or (_tl_ns(nc1) if timing else None)

    # host: assemble table1 + own views
    g1 = [np.asarray(r1.results[c]["g1s"]) for c in range(NCORES)]
    tab1 = np.zeros((TABR, F), np.float16)
    t1v = tab1.reshape(NCHUNK, NCORES, CSH, F)
    for c in range(NCORES):
        t1v[:, c] = g1[c].reshape(NCHUNK, CSH, F).astype(np.float16)
    tab1[prep["unocc_rows"]] = 0.0

    def own_view(gs):
        return np.ascontiguousarray(gs.reshape(128, WPC * F))

    def im_agg(c, tabname, tabv, ownv, extra):
        im = {
            tabname: tabv, "dloc": prep["dloc2d"][c], "norm": prep["norm2d"][c],
        }
        for ch in range(NCHUNK):
            gc = int(prep["Gc"][ch]) * 8
            im["idx%d" % ch] = np.ascontiguousarray(
                prep["idx2d"][c, ch][:, :gc])
        im.update(extra)
        im["own1" if tabname == "tab1" else "own2"] = ownv
        return im

    # ---- launch 2: agg conv1 + D2 -> g2s shards -------------------------
    nc2 = _build_launch2(prep)
    in_maps2 = []
    for c in range(NCORES):
        pk2 = _packed([
            np.ascontiguousarray(prep["dinv2"][c], np.float32),
            iotac,
            np.asarray(b_c1, np.float32).reshape(128, 1),
            np.asarray(b_fc2, np.float32).reshape(128, 1),
            iota, _np16(W_fc2), _np16(W_c2),
        ])
        in_maps2.append(im_agg(c, "tab1", tab1, own_view(g1[c]), {"pk": pk2}))
    r2 = run_bass_kernel_spmd(nc2, in_maps2, core_ids=list(range(NCORES)),
                              trace=trace)
    t2 = r2.exec_time_ns or (_tl_ns(nc2) if timing else None)

    g2 = [np.asarray(r2.results[c]["g2s"]) for c in range(NCORES)]
    tab2 = np.zeros((TABR, F), np.float16)
    t2v = tab2.reshape(NCHUNK, NCORES, CSH, F)
    for c in range(NCORES):
        t2v[:, c] = g2[c].reshape(NCHUNK, CSH, F).astype(np.float16)
    tab2[prep["unocc_rows"]] = 0.0

    # ---- launch 3: agg conv2 + D3 + pool --------------------------------
    nc3 = _build_launch3(prep)
    wfcp = np.zeros((128, NOUT), np.float16)
    wfcp[:] = _np16(W_fc)
    in_maps3 = []
    for c in range(NCORES):
        pk3 = _packed([
            np.ascontiguousarray(prep["dinv2"][c], np.float32),
            iotac,
            np.asarray(b_c2, np.float32).reshape(128, 1),
            iota, wfcp,
        ])
        in_maps3.append(im_agg(c, "tab2", tab2, own_view(g2[c]), {
            "pk": pk3, "g2d": np.ascontiguousarray(prep["g2d"][c])}))
    r3 = run_bass_kernel_spmd(nc3, in_maps3, core_ids=list(range(NCORES)),
                              trace=trace)
    t3 = r3.exec_time_ns or (_tl_ns(nc3) if timing else None)

    out = np.zeros((NG, NOUT), np.float64)
    for c in range(NCORES):
        out += np.asarray(r3.results[c]["pool"]).astype(np.float64)
    out = out + np.asarray(b_fc, np.float64)[None, :]

    LAST_EXEC_NS = (t1 or 0) + (t2 or 0) + (t3 or 0)
    LAST_INFO = {"t1": t1, "t2": t2, "t3": t3, "GT": prep["GT"]}
    return out.astype(np.float32)
